# revision 1
# baseline (speedup 1.0000x reference)
"""Trainium2 Bass kernel for nn_Model4 (retrieval_knn).

Model: 3 l2-normalized feature streams -> 4 chained MultiheadAttention blocks
-> full = rt @ t_r.T -> per-group cosine logits [4, 256, 256].

Sharding (8 cores = 4 row-groups x 2 head-groups):
  core c = 2*g + j.  g in 0..3 owns rows R_g = [256g, 256g+256) (== final group g),
  j in 0..1 owns heads {2j, 2j+1} == feature columns [512j, 512j+512) of qkv space.

All activations are kept "feat-major" in SBUF: X.T as [feat(partition), rows(free)]
so every GEMM is a natural matmul without transposes (weights are host-transposed).
Attention uses transposed softmax (scoresT [S, L], no max subtraction -- scores are
~1e-3 magnitude) with column sums done via ones-vector matmuls on the PE.

Per MHA: K/V projections are computed S-sharded and AllGather'd across row-groups
(4-rank groups [[0,2,4,6],[1,3,5,7]]); attention context halves are exchanged
within the (g,*) pair (2-rank groups) before the (replicated) output projection.

Precision: weights + attention path in bf16 (fp32 PSUM accumulate); the l2-norm
statistics and final cosine/logits path stay in fp32(r).
"""
import sys

sys.path.insert(0, "/opt/trn_rl_repo")

import ml_dtypes
import numpy as np

import concourse.bass as bass  # noqa: F401
import concourse.tile as tile
import concourse.mybir as mybir
from concourse import bacc
from concourse.bass_utils import run_bass_kernel_spmd

E = 1024
P = 128
KO = E // P          # 8 feature chunks
RG = 256             # rows per group
NCORES = 8
PIECE = P * 4 * RG   # 131072 elements: [128,4,256] / [128,2,512] piece
F32 = mybir.dt.float32
F32R = mybir.dt.float32r
BF16 = mybir.dt.bfloat16
AF = mybir.ActivationFunctionType
GROUPS4 = [[0, 2, 4, 6], [1, 3, 5, 7]]   # gather S-shards across row-groups
GROUPS2 = [[0, 1], [2, 3], [4, 5], [6, 7]]  # exchange head halves within pair
EPS = 1e-8

_CACHE = {}


def build_nc():
    nc = bacc.Bacc("TRN2", target_bir_lowering=False, debug=False,
                   num_devices=NCORES)
    dram = {}

    def din(name, shape, dt=BF16):
        dram[name] = nc.dram_tensor(name, shape, dt, kind="ExternalInput").ap()

    # raw feature slices (feat-major, this core's 256 rows)
    din("x_text", [E, RG], F32)
    din("x_loc", [E, RG], F32)
    din("x_glob", [E, RG], F32)
    # full (replicated) projection weights, host-transposed to [in, out]
    for w in ("w_tl", "w_tg", "w_rep"):
        din(w, [E, E], F32R)
    for b in ("b_tl", "b_tg", "b_rep"):
        din(b, [E], F32)
    din("pos_l", [E], F32)
    din("pos_g", [E], F32)
    # per-MHA weights; q/k/v are this core's head-half [in, 512]
    for m in ("tl", "tg", "ff", "rt"):
        din(f"wq_{m}", [E, 512])
        din(f"wk_{m}", [E, 512])
        din(f"wv_{m}", [E, 512])
        din(f"wo_{m}", [E, E], F32R)
        din(f"bq_{m}", [512], F32)
        din(f"bk_{m}", [512], F32)
        din(f"bv_{m}", [512], F32)
        din(f"bo_{m}", [E], F32)

    out_logits = nc.dram_tensor("logits", [RG, RG], F32,
                                kind="ExternalOutput").ap()

    from contextlib import ExitStack
    with tile.TileContext(nc) as tc, ExitStack() as ctx:
        consts = ctx.enter_context(tc.tile_pool(name="consts", bufs=1))
        acts = ctx.enter_context(tc.tile_pool(name="acts", bufs=4))
        pers = ctx.enter_context(tc.tile_pool(name="pers", bufs=1))
        qps = ctx.enter_context(tc.tile_pool(name="qps", bufs=3))
        exps = ctx.enter_context(tc.tile_pool(name="exps", bufs=2))
        kpfp = ctx.enter_context(tc.tile_pool(name="kpfp", bufs=2))
        vpfp = ctx.enter_context(tc.tile_pool(name="vpfp", bufs=1))
        accs = ctx.enter_context(tc.tile_pool(name="accs", bufs=2))
        accfp = ctx.enter_context(tc.tile_pool(name="accfp", bufs=1))
        kvs = ctx.enter_context(tc.tile_pool(name="kvs", bufs=2))
        sqs = ctx.enter_context(tc.tile_pool(name="sqs", bufs=2))
        bcs = ctx.enter_context(tc.tile_pool(name="bcs", bufs=2))
        smalls = ctx.enter_context(tc.tile_pool(name="smalls", bufs=2))
        weights = ctx.enter_context(tc.tile_pool(name="weights", bufs=3))
        outs = ctx.enter_context(tc.tile_pool(name="outs", bufs=1))
        ps256 = ctx.enter_context(tc.tile_pool(name="ps256", bufs=3, space="PSUM"))
        ps512 = ctx.enter_context(tc.tile_pool(name="ps512", bufs=2, space="PSUM"))
        pssum = ctx.enter_context(tc.tile_pool(name="pssum", bufs=2, space="PSUM"))
        dram_p = ctx.enter_context(tc.tile_pool(name="dram_p", bufs=1, space="DRAM"))

        # ---------- constants ----------
        ones_cb = consts.tile([P, 1], BF16)
        nc.vector.memset(ones_cb, 1.0)
        # f32r ones for the fp32r norm path (memset can't write f32r)
        ones_cf = consts.tile([P, 1], F32)
        nc.vector.memset(ones_cf, 1.0)
        ones_col = consts.tile([P, 1], F32R)
        nc.vector.tensor_copy(ones_col, ones_cf)
        ones_rf = consts.tile([1, P], F32)
        nc.vector.memset(ones_rf, 1.0)
        ones_row = consts.tile([1, P], F32R)
        nc.vector.tensor_copy(ones_row, ones_rf)

        def load_bias_pp(name, n):
            """[n] dram -> [128, n//128] per-partition scalar layout."""
            t = consts.tile([P, n // P], F32, name=f"c_{name}")
            nc.sync.dma_start(t, dram[name].rearrange("(c p) -> p c", p=P))
            return t

        bias_pp = {}
        for nm in ("b_tl", "b_tg", "b_rep", "pos_l", "pos_g"):
            bias_pp[nm] = load_bias_pp(nm, E)
        for m in ("tl", "tg", "ff", "rt"):
            for bn in ("bq", "bk", "bv"):
                bias_pp[f"{bn}_{m}"] = load_bias_pp(f"{bn}_{m}", 512)
            bias_pp[f"bo_{m}"] = load_bias_pp(f"bo_{m}", E)

        # ---------- helpers ----------
        def load_w(name, half):
            """weight [1024, 512] (or half of [1024,1024]) -> [128,8,512]."""
            wdt = dram[name].dtype
            t = weights.tile([P, KO, 512], wdt, tag="w",
                             name=f"w_{name}_{half}",
                             padded_shape=[P, KO, 1024] if wdt == BF16 else None)
            src = dram[name]
            if src.shape[1] == E:
                src = src[:, half * 512:(half + 1) * 512]
            nc.sync.dma_start(t, src.rearrange("(ko p) c -> p ko c", p=P))
            return t

        def bcast_row(row_f32r, n):
            """[1, n] f32r -> [128, n] f32 broadcast via K=1 outer product."""
            ps = ps256.tile([P, n], F32, tag="mm", name="ps_bc")
            nc.tensor.matmul(ps, ones_row, row_f32r, start=True, stop=True)
            out = bcs.tile([P, n], F32, tag=f"bc{n}", name="bc")
            nc.any.tensor_copy(out=out, in_=ps)
            return out

        def gemm_fm(w_tiles, act, out, nco, bias=None, residual=None):
            """Feat-major GEMM: out[:, c, :] = sum_ko w[:, ko, c-chunk].T @ act[:, ko]
            w_tiles: list of [128, 8, 512] bf16 tiles covering nco*128 chans.
            act [128, 8, R] bf16; out [128, nco, R]; bias [128, nco] f32."""
            R = act.shape[2]
            for c in range(nco):
                w_sb = w_tiles[c // 4]
                cc = c % 4
                ps = ps256.tile([P, R], F32, tag="mm", name="ps_g")
                for ko in range(KO):
                    nc.tensor.matmul(ps, w_sb[:, ko, cc * P:(cc + 1) * P],
                                     act[:, ko], start=(ko == 0),
                                     stop=(ko == KO - 1))
                if bias is not None:
                    nc.vector.tensor_scalar_add(out[:, c], ps, bias[:, c:c + 1])
                    if residual is not None:
                        nc.vector.tensor_add(out[:, c], out[:, c],
                                             residual[:, c])
                elif residual is not None:
                    nc.vector.tensor_add(out[:, c], ps, residual[:, c])
                else:
                    nc.any.tensor_copy(out=out[:, c], in_=ps)

        def colsum_inv(src, nko, with_sqrt_eps=False):
            """src [128, nko, R]: per-free-column 1/||col||; returns [1, R] f32r."""
            R = src.shape[2]
            ps = pssum.tile([1, R], F32, tag="cs", name="ps_cs")
            for ko in range(nko):
                sq = sqs.tile([P, R], F32R, tag="sq", name="sq")
                nc.vector.tensor_mul(sq, src[:, ko].bitcast(F32),
                                     src[:, ko].bitcast(F32))
                nc.tensor.matmul(ps, ones_col, sq, start=(ko == 0),
                                 stop=(ko == nko - 1))
            inv = smalls.tile([1, R], F32R, tag="inv", name="inv")
            norm = smalls.tile([1, R], F32, tag="nrm", name="nrm")
            nc.scalar.sqrt(norm, ps)
            if with_sqrt_eps:
                nc.vector.tensor_scalar_max(norm, norm, EPS)
            with nc.allow_low_precision(reason="fp32r rounding intended"):
                nc.vector.reciprocal(inv, norm)
            return inv

        def attention(qp, kpf, vpf, acc_out, bv_pp):
            """qp [128,4,256] bf16; kpf [128,4(gs),4(dc),256] bf16;
            vpf [128,4(gs),2(sc),512] bf16; acc_out [128,4,256] bf16."""
            for h in range(2):
                expt = exps.tile([P, KO, RG], F32R, tag="exp", name=f"expt{h}")
                pss = pssum.tile([1, RG], F32, tag="cs", name="ps_sm")
                for s in range(8):
                    ps = ps256.tile([P, RG], F32, tag="mm", name="ps_sc")
                    for dk in range(2):
                        nc.tensor.matmul(
                            ps,
                            kpf[:, s // 2, 2 * h + dk,
                                (s % 2) * P:(s % 2 + 1) * P],
                            qp[:, 2 * h + dk],
                            start=(dk == 0), stop=(dk == 1))
                    nc.scalar.activation(expt[:, s], ps, AF.Exp, scale=0.0625)
                for s in range(8):
                    nc.tensor.matmul(pss, ones_col, expt[:, s],
                                     start=(s == 0), stop=(s == 7))
                inv = smalls.tile([1, RG], F32R, tag="inv", name="inv_sm")
                with nc.allow_low_precision(reason="fp32r rounding intended"):
                    nc.vector.reciprocal(inv, pss)
                bc = bcast_row(inv, RG)
                for dk in range(2):
                    ps = ps256.tile([P, RG], F32, tag="mm", name="ps_av")
                    for s in range(8):
                        nc.tensor.matmul(
                            ps,
                            vpf[:, s // 2, s % 2,
                                256 * h + P * dk:256 * h + P * (dk + 1)],
                            expt[:, s],
                            start=(s == 0), stop=(s == 7))
                    nc.vector.tensor_mul(acc_out[:, 2 * h + dk], ps, bc)
                    nc.vector.tensor_scalar_add(
                        acc_out[:, 2 * h + dk], acc_out[:, 2 * h + dk],
                        bv_pp[:, 2 * h + dk:2 * h + dk + 1])

        def kv_project(m, kv_src):
            """returns (kp [128,4,256] bf16, vp [128,2,512] bf16)."""
            wk = load_w(f"wk_{m}", 0)
            kp = kvs.tile([P, 4, RG], BF16, tag="kp", name=f"kp_{m}")
            gemm_fm([wk], kv_src, kp, 4, bias=bias_pp[f"bk_{m}"])
            wv = load_w(f"wv_{m}", 0)
            vp = kvs.tile([P, 2, 512], F32R, tag="vp", name=f"vp_{m}")
            for mc in range(2):
                ps = ps512.tile([P, 512], F32, tag="mm512", name="ps_vp")
                for ko in range(KO):
                    nc.tensor.matmul(ps, kv_src[:, ko, mc * P:(mc + 1) * P],
                                     wv[:, ko], start=(ko == 0),
                                     stop=(ko == KO - 1))
                nc.any.tensor_copy(out=vp[:, mc], in_=ps)
            return kp, vp

        def pack_piece(inbuf, off, sb_tile):
            if sb_tile.dtype == BF16 and inbuf.dtype != BF16:
                sb_tile = sb_tile.bitcast(F32R)
            shp = sb_tile.shape
            n = P * shp[1] * shp[2]
            nc.sync.dma_start(
                inbuf[off:off + n].rearrange("(p a b) -> p a b", p=P,
                                             a=shp[1]), sb_tile)

        def allgather(inbuf, outbuf, groups):
            nc.gpsimd.collective_compute(
                "AllGather", mybir.AluOpType.bypass,
                replica_groups=groups,
                ins=[inbuf.opt()], outs=[outbuf.opt()])

        def load_kv_full(outbuf, kp_off, vp_off, m):
            # kp piece: bf16 stored as f32r pairs (PIECE//2 f32r elems);
            # vp piece: native f32r (PIECE elems)
            kpf = kpfp.tile([P, 4, 4, RG], BF16, tag="kpf", name=f"kpf_{m}")
            vpf = vpfp.tile([P, 4, 2, 512], F32R, tag="vpf", name=f"vpf_{m}")
            for gs in range(4):
                nc.sync.dma_start(
                    kpf[:, gs].bitcast(F32R),
                    outbuf[gs, kp_off:kp_off + PIECE // 2].rearrange(
                        "(p a b) -> p a b", p=P, a=4))
                nc.sync.dma_start(
                    vpf[:, gs],
                    outbuf[gs, vp_off:vp_off + PIECE].rearrange(
                        "(p a b) -> p a b", p=P, a=2))
            return kpf, vpf

        def out_proj(m, outbuf2, acc_off, residual, out_tile):
            accf = accfp.tile([P, KO, RG], F32R, tag="accf", name=f"accf_{m}")
            for pos in range(2):
                nc.sync.dma_start(
                    accf[:, pos * 4:(pos + 1) * 4],
                    outbuf2[pos, acc_off:acc_off + PIECE].rearrange(
                        "(p a b) -> p a b", p=P, a=4))
            wo = [load_w(f"wo_{m}", 0), load_w(f"wo_{m}", 1)]
            gemm_fm(wo, accf, out_tile, 8, bias=bias_pp[f"bo_{m}"],
                    residual=residual)

        # ---------- stage 0: load + normalize ----------
        def load_raw(name):
            t = acts.tile([P, KO, RG], F32, tag="act", name=f"raw_{name}")
            nc.sync.dma_start(t, dram[name].rearrange("(ko p) r -> p ko r",
                                                      p=P))
            return t

        textT = load_raw("x_text")
        locT = load_raw("x_loc")
        globT = load_raw("x_glob")

        def normalize(raw, out, pos_pp=None):
            inv = colsum_inv(raw, KO)
            bc = bcast_row(inv, RG)
            for ko in range(KO):
                nc.vector.tensor_mul(out[:, ko], raw[:, ko], bc)
                if pos_pp is not None:
                    nc.vector.tensor_scalar_add(out[:, ko], out[:, ko],
                                                pos_pp[:, ko:ko + 1])

        # textn: f32r master (t_x GEMMs) + bf16 copy (q/k/v projections)
        textn = acts.tile([P, KO, RG], F32R, tag="act", name="textn")
        normalize(textT, textn)
        textn_bf = acts.tile([P, KO, RG], BF16, tag="actb", name="textn_bf")
        for ko in range(KO):
            nc.vector.tensor_copy(textn_bf[:, ko], textn[:, ko])
        localn = pers.tile([P, KO, RG], F32R, name="localn")
        normalize(locT, localn)
        kvl = acts.tile([P, KO, RG], BF16, tag="actb", name="kvl")
        for ko in range(KO):
            nc.vector.tensor_scalar_add(kvl[:, ko], localn[:, ko].bitcast(F32),
                                        bias_pp["pos_l"][:, ko:ko + 1])
        kvg = acts.tile([P, KO, RG], BF16, tag="actb", name="kvg")
        normalize(globT, kvg, pos_pp=bias_pp["pos_g"])

        # ---------- stage A: text projections ----------
        qp_tl = qps.tile([P, 4, RG], BF16, tag="qp", name="qp_tl")
        gemm_fm([load_w("wq_tl", 0)], textn_bf, qp_tl, 4, bias=bias_pp["bq_tl"])
        qp_tg = qps.tile([P, 4, RG], BF16, tag="qp", name="qp_tg")
        gemm_fm([load_w("wq_tg", 0)], textn_bf, qp_tg, 4, bias=bias_pp["bq_tg"])
        t_l = acts.tile([P, KO, RG], F32, tag="act", name="t_l")
        gemm_fm([load_w("w_tl", 0), load_w("w_tl", 1)], textn, t_l, 8,
                bias=bias_pp["b_tl"])
        t_g = acts.tile([P, KO, RG], F32, tag="act", name="t_g")
        gemm_fm([load_w("w_tg", 0), load_w("w_tg", 1)], textn, t_g, 8,
                bias=bias_pp["b_tg"])
        # t_r: f32r master (AG piece + fullT lhsT); bf16 copy for qp_rt
        t_r = acts.tile([P, KO, RG], F32R, tag="act", name="t_r")
        gemm_fm([load_w("w_rep", 0), load_w("w_rep", 1)], textn, t_r, 8,
                bias=bias_pp["b_rep"])
        t_r_bf = acts.tile([P, KO, RG], BF16, tag="actb", name="t_r_bf")
        for ko in range(KO):
            nc.vector.tensor_copy(t_r_bf[:, ko], t_r[:, ko])

        # ---------- stage B: tl + tg MHAs ----------
        kp_tl, vp_tl = kv_project("tl", kvl)
        kp_tg, vp_tg = kv_project("tg", kvg)
        in1 = dram_p.tile([3 * PIECE], F32R, name="in1")
        out1 = dram_p.tile([4, 3 * PIECE], F32R, name="out1")
        pack_piece(in1, 0, kp_tl)                      # PIECE//2
        pack_piece(in1, PIECE // 2, vp_tl)             # PIECE
        pack_piece(in1, 3 * PIECE // 2, kp_tg)         # PIECE//2
        pack_piece(in1, 2 * PIECE, vp_tg)              # PIECE
        allgather(in1, out1, GROUPS4)

        kpf_tl, vpf_tl = load_kv_full(out1, 0, PIECE // 2, "tl")
        acc_tl = accs.tile([P, 4, RG], F32R, tag="acc", name="acc_tl")
        attention(qp_tl, kpf_tl, vpf_tl, acc_tl, bias_pp["bv_tl"])
        kpf_tg, vpf_tg = load_kv_full(out1, 3 * PIECE // 2, 2 * PIECE, "tg")
        acc_tg = accs.tile([P, 4, RG], F32R, tag="acc", name="acc_tg")
        attention(qp_tg, kpf_tg, vpf_tg, acc_tg, bias_pp["bv_tg"])

        in2 = dram_p.tile([2 * PIECE], F32R, name="in2")
        out2 = dram_p.tile([2, 2 * PIECE], F32R, name="out2")
        pack_piece(in2, 0, acc_tl)
        pack_piece(in2, PIECE, acc_tg)
        allgather(in2, out2, GROUPS2)

        # lt / ff have residual uses -> keep f32 master + bf16 GEMM copy
        lt = acts.tile([P, KO, RG], F32, tag="act", name="lt")
        out_proj("tl", out2, 0, t_l, lt)
        gt = acts.tile([P, KO, RG], BF16, tag="actb", name="gt")
        out_proj("tg", out2, PIECE, t_g, gt)
        lt_bf = acts.tile([P, KO, RG], BF16, tag="actb", name="lt_bf")
        for ko in range(KO):
            nc.vector.tensor_copy(lt_bf[:, ko], lt[:, ko])

        # ---------- stage C: ff MHA (q=lt, kv=gt) ----------
        qp_ff = qps.tile([P, 4, RG], BF16, tag="qp", name="qp_ff")
        gemm_fm([load_w("wq_ff", 0)], lt_bf, qp_ff, 4, bias=bias_pp["bq_ff"])
        kp_ff, vp_ff = kv_project("ff", gt)
        in3 = dram_p.tile([3 * PIECE // 2], F32R, name="in3")
        out3 = dram_p.tile([4, 3 * PIECE // 2], F32R, name="out3")
        pack_piece(in3, 0, kp_ff)
        pack_piece(in3, PIECE // 2, vp_ff)
        allgather(in3, out3, GROUPS4)

        kpf_ff, vpf_ff = load_kv_full(out3, 0, PIECE // 2, "ff")
        acc_ff = accs.tile([P, 4, RG], F32R, tag="acc", name="acc_ff")
        attention(qp_ff, kpf_ff, vpf_ff, acc_ff, bias_pp["bv_ff"])
        in4 = dram_p.tile([PIECE], F32R, name="in4")
        out4 = dram_p.tile([2, PIECE], F32R, name="out4")
        pack_piece(in4, 0, acc_ff)
        allgather(in4, out4, GROUPS2)
        ff = acts.tile([P, KO, RG], BF16, tag="actb", name="ff")
        out_proj("ff", out4, 0, lt, ff)

        # ---------- stage D: rt MHA (q=t_r, kv=ff) ----------
        qp_rt = qps.tile([P, 4, RG], BF16, tag="qp", name="qp_rt")
        gemm_fm([load_w("wq_rt", 0)], t_r_bf, qp_rt, 4, bias=bias_pp["bq_rt"])
        kp_rt, vp_rt = kv_project("rt", ff)
        in5 = dram_p.tile([7 * PIECE // 2], F32R, name="in5")
        out5 = dram_p.tile([4, 7 * PIECE // 2], F32R, name="out5")
        pack_piece(in5, 0, kp_rt)                     # PIECE//2
        pack_piece(in5, PIECE // 2, vp_rt)            # PIECE
        pack_piece(in5, 3 * PIECE // 2, t_r)          # 2*PIECE
        allgather(in5, out5, GROUPS4)

        kpf_rt, vpf_rt = load_kv_full(out5, 0, PIECE // 2, "rt")
        acc_rt = accs.tile([P, 4, RG], F32R, tag="acc", name="acc_rt")
        attention(qp_rt, kpf_rt, vpf_rt, acc_rt, bias_pp["bv_rt"])
        in6 = dram_p.tile([PIECE], F32R, name="in6")
        out6 = dram_p.tile([2, PIECE], F32R, name="out6")
        pack_piece(in6, 0, acc_rt)
        allgather(in6, out6, GROUPS2)
        rt = acts.tile([P, KO, RG], F32R, tag="act", name="rt")
        out_proj("rt", out6, 0, None, rt)

        # ---------- stage E: full = rt @ t_r.T, cosine logits ----------
        fullT = acts.tile([P, KO, RG], F32, tag="act", name="fullT")
        for gs in range(4):
            trf = exps.tile([P, KO, RG], F32R, tag="exp", name=f"trf{gs}")
            nc.sync.dma_start(
                trf, out5[gs, 3 * PIECE // 2:7 * PIECE // 2].rearrange(
                    "(p a b) -> p a b", p=P, a=KO))
            for mh in range(2):
                mc = gs * 2 + mh
                ps = ps256.tile([P, RG], F32, tag="mm", name="ps_full")
                for ko in range(KO):
                    nc.tensor.matmul(ps, trf[:, ko, mh * P:(mh + 1) * P],
                                     rt[:, ko], start=(ko == 0),
                                     stop=(ko == KO - 1))
                nc.any.tensor_copy(out=fullT[:, mc], in_=ps)

        inv_full = colsum_inv(fullT, KO, with_sqrt_eps=True)
        bc_full = bcast_row(inv_full, RG)
        ffn = acts.tile([P, KO, RG], F32R, tag="act", name="ffn")
        for ko in range(KO):
            nc.vector.tensor_mul(ffn[:, ko], fullT[:, ko], bc_full)

        lg = outs.tile([P, 2, RG], F32, name="lg")
        for lc in range(2):
            ps = ps256.tile([P, RG], F32, tag="mm", name="ps_lg")
            for ko in range(KO):
                nc.tensor.matmul(ps, ffn[:, ko, lc * P:(lc + 1) * P],
                                 localn[:, ko], start=(ko == 0),
                                 stop=(ko == KO - 1))
            nc.any.tensor_copy(out=lg[:, lc], in_=ps)
        nc.sync.dma_start(out_logits.rearrange("(lc p) q -> p lc q", p=P), lg)

    nc.compile()
    return nc


def make_in_maps(local_feat, global_feat, text_feat,
                 w_tl, b_tl, w_tg, b_tg, w_rep, b_rep,
                 pos_local, pos_global, mha_params):
    """mha_params: dict m -> (wi, bi, wo, bo)."""
    f32 = np.float32
    bf16 = ml_dtypes.bfloat16
    textT = np.ascontiguousarray(text_feat.T.astype(f32))
    locT = np.ascontiguousarray(local_feat.T.astype(f32))
    globT = np.ascontiguousarray(global_feat.T.astype(f32))
    shared = {
        "w_tl": np.ascontiguousarray(w_tl.T.astype(f32)),
        "w_tg": np.ascontiguousarray(w_tg.T.astype(f32)),
        "w_rep": np.ascontiguousarray(w_rep.T.astype(f32)),
        "b_tl": b_tl.astype(f32), "b_tg": b_tg.astype(f32),
        "b_rep": b_rep.astype(f32),
        "pos_l": pos_local.astype(f32), "pos_g": pos_global.astype(f32),
    }
    per_j = {}
    for j in range(2):
        d = {}
        for m, (wi, bi, wo, bo) in mha_params.items():
            sl = slice(512 * j, 512 * (j + 1))
            d[f"wq_{m}"] = np.ascontiguousarray(wi[0 * E:1 * E][sl].T.astype(bf16))
            d[f"wk_{m}"] = np.ascontiguousarray(wi[1 * E:2 * E][sl].T.astype(bf16))
            d[f"wv_{m}"] = np.ascontiguousarray(wi[2 * E:3 * E][sl].T.astype(bf16))
            d[f"wo_{m}"] = np.ascontiguousarray(wo.T.astype(f32))
            d[f"bq_{m}"] = bi[0 * E:1 * E][sl].astype(f32)
            d[f"bk_{m}"] = bi[1 * E:2 * E][sl].astype(f32)
            d[f"bv_{m}"] = bi[2 * E:3 * E][sl].astype(f32)
            d[f"bo_{m}"] = bo.astype(f32)
        per_j[j] = d

    in_maps = []
    for c in range(NCORES):
        g, j = c // 2, c % 2
        rs = slice(RG * g, RG * (g + 1))
        m = {
            "x_text": np.ascontiguousarray(textT[:, rs]),
            "x_loc": np.ascontiguousarray(locT[:, rs]),
            "x_glob": np.ascontiguousarray(globT[:, rs]),
        }
        m.update(shared)
        m.update(per_j[j])
        in_maps.append(m)
    return in_maps


def kernel(local_feat, global_feat, text_feat,
           w_tl, b_tl, w_tg, b_tg, w_rep, b_rep,
           pos_local, pos_global,
           tl_wi, tl_bi, tl_wo, tl_bo,
           tg_wi, tg_bi, tg_wo, tg_bo,
           ff_wi, ff_bi, ff_wo, ff_bo,
           rt_wi, rt_bi, rt_wo, rt_bo,
           n_groups):
    assert int(n_groups) == 4
    if "nc" not in _CACHE:
        _CACHE["nc"] = build_nc()
    nc = _CACHE["nc"]
    mha_params = {
        "tl": (tl_wi, tl_bi, tl_wo, tl_bo),
        "tg": (tg_wi, tg_bi, tg_wo, tg_bo),
        "ff": (ff_wi, ff_bi, ff_wo, ff_bo),
        "rt": (rt_wi, rt_bi, rt_wo, rt_bo),
    }
    in_maps = make_in_maps(np.asarray(local_feat), np.asarray(global_feat),
                           np.asarray(text_feat),
                           np.asarray(w_tl), np.asarray(b_tl),
                           np.asarray(w_tg), np.asarray(b_tg),
                           np.asarray(w_rep), np.asarray(b_rep),
                           np.asarray(pos_local), np.asarray(pos_global),
                           {k: tuple(np.asarray(x) for x in v)
                            for k, v in mha_params.items()})
    res = run_bass_kernel_spmd(nc, in_maps, core_ids=list(range(NCORES)))
    _CACHE["last_results"] = res
    out = np.empty((4, RG, RG), dtype=np.float32)
    for g in range(4):
        out[g] = res.results[2 * g]["logits"]
    return out



# revision 17
# speedup vs baseline: 1.9125x; 1.9125x over previous
"""Trainium2 Bass kernel for nn_Model4 (retrieval_knn).

Model: 3 l2-normalized feature streams -> 4 chained MultiheadAttention blocks
-> full = rt @ t_r.T -> per-group cosine logits [4, 256, 256].

Sharding (v2): 8-way row sharding (core c owns rows [128c, 128c+128)) with
REPLICATED K/V projections.  K/V sources for the tl/tg MHAs (local_n+pos,
global_n+pos) derive from inputs, so every core computes full-sequence K/V
locally; only the two intermediate activations that cross MHAs (gt, ff) are
AllGather'd.  That cuts the collective count from 6 (614us) to 2 (135us), and
both gathers overlap with independent compute (tl-MHA during the gt gather,
t_r/final-stage prep during the ff gather).

Bias algebra: K-projection bias drops out of softmax (adds a per-query
constant to every score); V-projection bias is folded into the output
projection bias host-side (bo_eff = bo + wo @ bv).

Layouts: activations feat-major ([chan(part) x chunks, rows(free)]); V
projections row-major ([S(part) x chunks, dv(free)]) so they serve as AV
lhsT directly; attention uses transposed-softmax (no max subtraction; scores
are ~1e-3).  Gathered tensors keep rank-major S order == global row order.
The full-text normalization scale commutes through the t_r GEMM (columns
scaled post-GEMM), so normalized full text is never materialized.
"""
import sys

sys.path.insert(0, "/opt/trn_rl_repo")

import ml_dtypes
import numpy as np

import concourse.bass as bass  # noqa: F401
import concourse.tile as tile
import concourse.mybir as mybir
from concourse import bacc
from concourse.bass_utils import run_bass_kernel_spmd
from concourse.masks import make_identity

E = 1024
P = 128
KO = 8               # feature chunks of 128
L = 128              # rows per core
GRP = 256            # rows per output group
NCORES = 8
F32 = mybir.dt.float32
F32R = mybir.dt.float32r
BF16 = mybir.dt.bfloat16
AF = mybir.ActivationFunctionType
GROUPS8 = [[0, 1, 2, 3, 4, 5, 6, 7]]
EPS = 1e-8
PIECE = P * KO * L   # 131072 bf16 elements in one packed [128,8,128] piece

DEBUG = False
_CACHE = {}


def build_nc():
    nc = bacc.Bacc("TRN2", target_bir_lowering=False, debug=False,
                   num_devices=NCORES)
    dram = {}

    def din(name, shape, dt=BF16):
        dram[name] = nc.dram_tensor(name, shape, dt, kind="ExternalInput").ap()

    # full feat-major feature streams (replicated), host-converted to bf16
    din("x_glob", [E, E])
    din("x_text", [E, E])
    din("x_loc", [E, E])
    # per-core slices
    din("x_text_own", [E, L])
    din("x_loc_grp", [E, GRP])
    # shared projections, host-transposed to [cin, cout]
    for w in ("w_tl", "w_tg", "w_rep"):
        din(w, [E, E])
    for b in ("b_tl", "b_tg", "b_rep"):
        din(b, [E], F32)
    din("pos_l", [E], F32)
    din("pos_g", [E], F32)
    # per-MHA weights, host-transposed to [cin, cout]; K bias dropped,
    # V bias folded into bo host-side.
    for m in ("tl", "tg", "ff", "rt"):
        for w in ("wq", "wk", "wv", "wo"):
            din(f"{w}_{m}", [E, E])
        din(f"bq_{m}", [E], F32)
        din(f"bo_{m}", [E], F32)

    out_logits = nc.dram_tensor("logits", [L, GRP], F32,
                                kind="ExternalOutput").ap()
    dbg = {}
    if DEBUG:
        for nm, shape, dt in [
                ("d_kvg", [E, E], BF16), ("d_textn_own", [E, L], BF16),
                ("d_t_g", [E, L], BF16), ("d_qp_tg", [E, L], BF16),
                ("d_kp_tg", [E, E], BF16), ("d_vp_tg", [E, E], BF16),
                ("d_ctx_tg", [L, E], BF16), ("d_gt", [E, L], BF16),
                ("d_gtf", [KO, E, L], BF16), ("d_lt", [E, L], F32),
                ("d_ff", [E, L], BF16), ("d_rt", [E, L], BF16),
                ("d_t_r", [E, E], BF16), ("d_frow", [L, E], BF16),
                ("d_lfn", [E, GRP], BF16)]:
            dbg[nm] = nc.dram_tensor(nm, shape, dt,
                                     kind="ExternalOutput").ap()

    from contextlib import ExitStack
    with tile.TileContext(nc) as tc, ExitStack() as ctx:
        def pool(name, bufs, space="SBUF"):
            return ctx.enter_context(
                tc.tile_pool(name=name, bufs=bufs, space=space))

        consts = pool("consts", 1)
        raws = pool("raws", 1)       # one big raw stream buffer (16k)
        rawsm = pool("rawsm", 1)     # small raw slices (3k)
        acts = pool("acts", 1)       # textn_own (2k)
        kvsrc = pool("kvsrc", 1)     # kvg/kvl (16k)
        gath = pool("gath", 1)       # gathered gt/ff (16k)
        kps = pool("kps", 1)         # K proj (16k)
        vps = pool("vps", 1)         # V proj (16k)
        exps = pool("exps", 1)       # exp scores (8k)
        ctxs = pool("ctxs", 1)       # ctx + ctxT (4k)
        pers = pool("pers", 1)       # t_r (16k) + lfn (4k)
        bcs = pool("bcs", 1)         # broadcast tiles (~10k)
        smalls = pool("smalls", 1)   # inv/nrm rows (~10k)
        finals = pool("finals", 1)   # final-stage tiles (~5k)
        sqs = pool("sqs", 2)         # squared chunks (2k x2)
        pers2 = pool("pers2", 2)     # f32 masters t_g/t_l/lt (12k)
        qps = pool("qps", 2)         # q projections (4k)
        outs_p = pool("outs_p", 2)   # MHA outputs pre-pack (4k)
        weights = pool("weights", 2)  # streamed weights (32k)
        psA = pool("psA", 4, space="PSUM")
        pssum = pool("pssum", 2, space="PSUM")
        psT = pool("psT", 2, space="PSUM")
        dram_p = pool("dram_p", 1, space="DRAM")

        # ---------- constants ----------
        ones_cb = consts.tile([P, 1], BF16)
        nc.vector.memset(ones_cb, 1.0)
        ones_cf32 = consts.tile([P, 1], F32)
        nc.vector.memset(ones_cf32, 1.0)
        ones_cr = consts.tile([P, 1], F32R)
        nc.vector.tensor_copy(ones_cr, ones_cf32)
        ones_rf32 = consts.tile([1, P], F32)
        nc.vector.memset(ones_rf32, 1.0)
        ones_rr = consts.tile([1, P], F32R)
        nc.vector.tensor_copy(ones_rr, ones_rf32)
        ident = consts.tile([P, P], BF16)
        make_identity(nc, ident)

        def load_bias_pp(name):
            t = consts.tile([P, KO], F32, name=f"c_{name}")
            nc.sync.dma_start(t, dram[name].rearrange("(c p) -> p c", p=P))
            return t

        bias_pp = {}
        for nm in ("b_tl", "b_tg", "b_rep", "pos_l", "pos_g"):
            bias_pp[nm] = load_bias_pp(nm)
        for m in ("tl", "tg", "ff", "rt"):
            bias_pp[f"bq_{m}"] = load_bias_pp(f"bq_{m}")
            bias_pp[f"bo_{m}"] = load_bias_pp(f"bo_{m}")

        # ---------- helpers ----------
        def load_w(name):
            """[1024, 1024] bf16 dram -> [128, 8, 1024] (p, cin-chunk, cout)."""
            t = weights.tile([P, KO, E], BF16, tag="w", name=f"w_{name}")
            nc.sync.dma_start(t, dram[name].rearrange("(ko p) c -> p ko c",
                                                      p=P))
            return t

        def norm_inv(raw, n, ncols=None):
            """Per-free-column 1/l2norm over all 8 chunks of raw [128,8,n].

            Squares on Act in 512-wide chunks -> f32r, partition-summed via
            ones-matmuls, sqrt + reciprocal.  Returns [1, n] f32r.
            """
            ncols = ncols or n
            nhalf = (ncols + 511) // 512
            inv = smalls.tile([1, ncols], F32R, tag="inv", name="inv",
                              padded_shape=[1, E])
            nrm = smalls.tile([1, ncols], F32, tag="nrm", name="nrm",
                              padded_shape=[1, E])
            pss = [pssum.tile([1, min(512, ncols)], F32, tag="cs",
                              name="ps_cs") for _ in range(nhalf)]
            for ko in range(KO):
                for h in range(nhalf):
                    lo = h * 512
                    hi = min(ncols, lo + 512)
                    sq = sqs.tile([P, 512], F32R, tag="sq", name="sq",
                                  padded_shape=[P, 512])
                    nc.scalar.activation(sq[:, :hi - lo], raw[:, ko, lo:hi],
                                         AF.Square)
                    nc.tensor.matmul(pss[h][:, :hi - lo], ones_cr,
                                     sq[:, :hi - lo], start=(ko == 0),
                                     stop=(ko == KO - 1))
            for h in range(nhalf):
                lo = h * 512
                hi = min(ncols, lo + 512)
                nc.scalar.sqrt(nrm[:, lo:hi], pss[h][:, :hi - lo])
            with nc.allow_low_precision(reason="norm reciprocal"):
                nc.vector.reciprocal(inv, nrm)
            return inv

        def bcast_row(row_r, n, dtype, tag, name="bc"):
            """[1, n] f32r -> [128, n] broadcast tile."""
            out = bcs.tile([P, n], dtype, tag=tag, name=name)
            for h in range((n + 511) // 512):
                lo = h * 512
                hi = min(n, lo + 512)
                ps = psA.tile([P, 512], F32, tag="mm", name="ps_bc")
                nc.tensor.matmul(ps[:, :hi - lo], ones_rr, row_r[:, lo:hi],
                                 start=True, stop=True)
                nc.scalar.activation(out[:, lo:hi], ps[:, :hi - lo], AF.Copy)
            return out

        def load_raw(xname, n, big):
            p = raws if big else rawsm
            raw = p.tile([P, KO, n], BF16, tag=f"raw{n}", name=f"raw_{xname}")
            for ko in range(KO):
                nc.sync.dma_start(raw[:, ko],
                                  dram[xname][ko * P:(ko + 1) * P, :])
            return raw

        def normalize(xname, n, out_pool, pos=None, tag=None, big=False):
            """bf16 feat-major [E, n] dram -> l2-normalized bf16 [128,8,n]
            (+ optional per-chan pos bias)."""
            raw = load_raw(xname, n, big)
            inv = norm_inv(raw, n)
            bc = bcast_row(inv, n, F32R, tag=f"bcn{n}", name=f"bc_{xname}")
            out = out_pool.tile([P, KO, n], BF16, tag=tag or f"nb{n}",
                                name=f"n_{xname}")
            for ko in range(KO):
                nc.vector.tensor_mul(out[:, ko], raw[:, ko], bc)
                if pos is not None:
                    nc.vector.tensor_scalar_add(out[:, ko], out[:, ko],
                                                pos[:, ko:ko + 1])
            return out

        def gemm_own(w_sb, src_bf, bias, name, residual=None, master=False,
                     out_pool=None):
            """Own-rows GEMM via fat row-major matmuls (lhsT = feat-major
            src), then PE-transpose back to feat-major [128, 8(co), 128(L)].
            16 fat matmuls + 8 transposes instead of 64 skinny matmuls."""
            pool_ = out_pool or outs_p
            out_bf = pool_.tile([P, KO, L], BF16, tag="ob", name=name)
            out_f = None
            if master:
                out_f = pool_.tile([P, KO, L], F32, tag="of", name=name + "_f")
            row = ctxs.tile([P, E], BF16, tag="grow", name=f"row_{name}")
            pss = [psA.tile([P, 512], F32, tag="mm", name=f"ps_go{h}")
                   for h in range(2)]
            for ci in range(KO):
                for h in range(2):
                    nc.tensor.matmul(pss[h], src_bf[:, ci],
                                     w_sb[:, ci, h * 512:(h + 1) * 512],
                                     start=(ci == 0), stop=(ci == KO - 1))
            for h in range(2):
                nc.scalar.activation(row[:, h * 512:(h + 1) * 512], pss[h],
                                     AF.Copy)
            for co in range(KO):
                pt = psT.tile([P, P], BF16, tag="tr", name="ps_gt")
                nc.tensor.transpose(pt, row[:, co * P:(co + 1) * P], ident)
                tgt = out_f if master else out_bf
                if bias is not None:
                    nc.vector.tensor_scalar_add(tgt[:, co], pt,
                                                bias[:, co:co + 1])
                    if residual is not None:
                        nc.vector.tensor_add(tgt[:, co], tgt[:, co],
                                             residual[:, co])
                elif residual is not None:
                    nc.vector.tensor_add(tgt[:, co], pt, residual[:, co])
                else:
                    nc.vector.tensor_copy(tgt[:, co], pt)
                if master:
                    nc.scalar.activation(out_bf[:, co], out_f[:, co], AF.Copy)
            return out_bf, out_f

        # kv source accessors: plain [128, 8(ci), 1024(S)] or gathered
        # [128, 8(rank), 8(ci), 128]
        def src_rhs(src, ci, h4):
            if len(src.shape) == 4:
                return src[:, h4 * 4:(h4 + 1) * 4, ci, :]
            return src[:, ci, h4 * 512:(h4 + 1) * 512]

        def src_lhsT(src, ci, s):
            if len(src.shape) == 4:
                return src[:, s, ci, :]
            return src[:, ci, s * P:(s + 1) * P]

        def kv_project(m, src):
            wk = load_w(f"wk_{m}")
            kp = kps.tile([P, KO, E], BF16, tag="kp", name=f"kp_{m}")
            for co in range(KO):
                for h4 in range(2):
                    ps = psA.tile([P, 512], F32, tag="mm", name="ps_k")
                    for ci in range(KO):
                        nc.tensor.matmul(ps, wk[:, ci, co * P:(co + 1) * P],
                                         src_rhs(src, ci, h4),
                                         start=(ci == 0), stop=(ci == KO - 1))
                    nc.scalar.activation(kp[:, co, h4 * 512:(h4 + 1) * 512],
                                         ps, AF.Copy)
            wv = load_w(f"wv_{m}")
            vp = vps.tile([P, KO, E], BF16, tag="vp", name=f"vp_{m}")
            for s in range(KO):
                for dh in range(2):
                    ps = psA.tile([P, 512], F32, tag="mm", name="ps_v")
                    for ci in range(KO):
                        nc.tensor.matmul(ps, src_lhsT(src, ci, s),
                                         wv[:, ci, dh * 512:(dh + 1) * 512],
                                         start=(ci == 0), stop=(ci == KO - 1))
                    nc.scalar.activation(vp[:, s, dh * 512:(dh + 1) * 512],
                                         ps, AF.Copy)
            return kp, vp

        def attention(m, qp, kp, vp):
            """-> ctxT [128, 8(ci), 128(L)] bf16 (pre-out-proj context)."""
            expt = exps.tile([P, KO, 512], BF16, tag="exp", name=f"expt_{m}")
            for s in range(KO):
                ps = psA.tile([P, 512], F32, tag="mm", name="ps_sc")
                for h in range(4):
                    for dk in range(2):
                        nc.tensor.matmul(
                            ps[:, h * P:(h + 1) * P],
                            kp[:, 2 * h + dk, s * P:(s + 1) * P],
                            qp[:, 2 * h + dk], start=(dk == 0), stop=(dk == 1))
                nc.scalar.activation(expt[:, s], ps, AF.Exp, scale=0.0625)
            pss = pssum.tile([1, 512], F32, tag="cs", name="ps_sm")
            for s in range(KO):
                nc.tensor.matmul(pss, ones_cb, expt[:, s], start=(s == 0),
                                 stop=(s == KO - 1))
            inv = smalls.tile([1, 512], F32R, tag="inv512", name="inv_sm")
            with nc.allow_low_precision(reason="softmax reciprocal"):
                nc.vector.reciprocal(inv, pss)
            bc = bcast_row(inv, 512, BF16, tag="bcs", name=f"bcs_{m}")
            for s in range(KO):
                nc.vector.tensor_mul(expt[:, s], expt[:, s], bc)
            ctx = ctxs.tile([P, E], BF16, tag="ctx", name=f"ctx_{m}")
            for hh in range(2):
                ps = psA.tile([P, 512], F32, tag="mm", name="ps_av")
                for hi in range(2):
                    h = 2 * hh + hi
                    for s in range(KO):
                        nc.tensor.matmul(
                            ps[:, hi * 256:(hi + 1) * 256],
                            expt[:, s, h * P:(h + 1) * P],
                            vp[:, s, h * 256:(h + 1) * 256],
                            start=(s == 0), stop=(s == KO - 1))
                nc.scalar.activation(ctx[:, hh * 512:(hh + 1) * 512], ps,
                                     AF.Copy)
            if DEBUG and m == "tg":
                nc.sync.dma_start(dbg["d_ctx_tg"], ctx)
            ctxT = ctxs.tile([P, KO, L], BF16, tag="ctxT", name=f"ctxT_{m}")
            for ci in range(KO):
                pt = psT.tile([P, P], BF16, tag="tr", name="ps_tr")
                nc.tensor.transpose(pt, ctx[:, ci * P:(ci + 1) * P], ident)
                nc.vector.tensor_copy(ctxT[:, ci], pt)
            return ctxT

        def out_proj(m, ctxT, residual, master=False, out_pool=None):
            wo = load_w(f"wo_{m}")
            return gemm_own(wo, ctxT, bias_pp[f"bo_{m}"], f"o_{m}",
                            residual=residual, master=master,
                            out_pool=out_pool)

        def dump_feat(nm, t):
            if DEBUG:
                nc.sync.dma_start(
                    dbg[nm].rearrange("(ko p) r -> p ko r", p=P), t)

        def dump_plain(nm, t):
            if DEBUG:
                nc.sync.dma_start(dbg[nm], t)

        def pack_piece(inbuf, sb_tile):
            # NB: collective buffers must be bf16/f32 -- f32r payloads get
            # mantissa-squashed by the collective transport in this runtime.
            nc.sync.dma_start(
                inbuf.rearrange("(p a b) -> p a b", p=P, a=KO), sb_tile)

        def allgather(inbuf, outbuf):
            nc.gpsimd.collective_compute(
                "AllGather", mybir.AluOpType.bypass,
                replica_groups=GROUPS8,
                ins=[inbuf.opt()], outs=[outbuf.opt()])

        def unpack_gather(outbuf, name):
            t = gath.tile([P, KO, KO, L], BF16, tag="gf", name=name)
            for r in range(KO):
                nc.sync.dma_start(
                    t[:, r],
                    outbuf[r].rearrange("(p a b) -> p a b", p=P, a=KO))
            return t

        # ---------- stage 0: normalize (own text first, then glob) ----------
        textn_own = normalize("x_text_own", L, acts, tag="nto")
        kvg = normalize("x_glob", E, kvsrc, pos=bias_pp["pos_g"], big=True)

        # ---------- tg path ----------
        w_tg = load_w("w_tg")
        t_g_bf, t_g = gemm_own(w_tg, textn_own, bias_pp["b_tg"], "t_g",
                               master=True, out_pool=pers2)
        wq_tg = load_w("wq_tg")
        qp_tg, _ = gemm_own(wq_tg, t_g_bf, bias_pp["bq_tg"], "qp_tg",
                            out_pool=qps)
        kp_tg, vp_tg = kv_project("tg", kvg)
        ctxT_tg = attention("tg", qp_tg, kp_tg, vp_tg)
        gt_bf, _ = out_proj("tg", ctxT_tg, t_g)
        dump_feat("d_kvg", kvg)
        dump_feat("d_textn_own", textn_own)
        dump_feat("d_t_g", t_g_bf)
        dump_feat("d_qp_tg", qp_tg)
        dump_feat("d_kp_tg", kp_tg)
        dump_feat("d_vp_tg", vp_tg)
        dump_feat("d_gt", gt_bf)

        in1 = dram_p.tile([PIECE], BF16, name="in1")
        out1 = dram_p.tile([KO, PIECE], BF16, name="out1")
        pack_piece(in1, gt_bf)
        allgather(in1, out1)

        # ---------- tl path (overlaps gt gather) ----------
        kvl = normalize("x_loc", E, kvsrc, pos=bias_pp["pos_l"], big=True)
        w_tl = load_w("w_tl")
        t_l_bf, t_l = gemm_own(w_tl, textn_own, bias_pp["b_tl"], "t_l",
                               master=True, out_pool=pers2)
        wq_tl = load_w("wq_tl")
        qp_tl, _ = gemm_own(wq_tl, t_l_bf, bias_pp["bq_tl"], "qp_tl",
                            out_pool=qps)
        kp_tl, vp_tl = kv_project("tl", kvl)
        ctxT_tl = attention("tl", qp_tl, kp_tl, vp_tl)
        lt_bf, lt = out_proj("tl", ctxT_tl, t_l, master=True, out_pool=pers2)
        wq_ff = load_w("wq_ff")
        qp_ff, _ = gemm_own(wq_ff, lt_bf, bias_pp["bq_ff"], "qp_ff",
                            out_pool=qps)
        # full-text norm scale (for t_r in the next window); the normalized
        # text itself is never materialized -- the scale commutes through
        # the t_r GEMM.
        raw_text = load_raw("x_text", E, big=True)
        inv_text = norm_inv(raw_text, E)
        bc_text = bcast_row(inv_text, E, F32R, tag="bct", name="bc_text")

        # ---------- ff MHA ----------
        gt_full = unpack_gather(out1, "gt_full")
        if DEBUG:
            for r in range(KO):
                nc.sync.dma_start(
                    dbg["d_gtf"][r].rearrange("(ko p) l -> p ko l", p=P),
                    gt_full[:, r])
        kp_ff, vp_ff = kv_project("ff", gt_full)
        ctxT_ff = attention("ff", qp_ff, kp_ff, vp_ff)
        ff_bf, _ = out_proj("ff", ctxT_ff, lt)
        dump_feat("d_lt", lt)
        dump_feat("d_ff", ff_bf)

        in2 = dram_p.tile([PIECE], BF16, name="in2")
        out2 = dram_p.tile([KO, PIECE], BF16, name="out2")
        pack_piece(in2, ff_bf)
        allgather(in2, out2)

        # ---------- window 2 (overlaps ff gather): t_r + final prep ----------
        w_rep = load_w("w_rep")
        t_r = pers.tile([P, KO, E], BF16, name="t_r")
        for co in range(KO):
            for h4 in range(2):
                ps = psA.tile([P, 512], F32, tag="mm", name="ps_tr2")
                for ci in range(KO):
                    nc.tensor.matmul(ps, w_rep[:, ci, co * P:(co + 1) * P],
                                     raw_text[:, ci, h4 * 512:(h4 + 1) * 512],
                                     start=(ci == 0), stop=(ci == KO - 1))
                sl = t_r[:, co, h4 * 512:(h4 + 1) * 512]
                nc.vector.tensor_mul(sl, ps, bc_text[:, h4 * 512:(h4 + 1) * 512])
                nc.vector.tensor_scalar_add(sl, sl,
                                            bias_pp["b_rep"][:, co:co + 1])
        t_r_own, _ = gemm_own(w_rep, textn_own, bias_pp["b_rep"], "t_r_own")
        wq_rt = load_w("wq_rt")
        qp_rt, _ = gemm_own(wq_rt, t_r_own, bias_pp["bq_rt"], "qp_rt",
                            out_pool=qps)
        lfn = normalize("x_loc_grp", GRP, pers, tag="lfn")

        # ---------- rt MHA ----------
        ff_full = unpack_gather(out2, "ff_full")
        kp_rt, vp_rt = kv_project("rt", ff_full)
        ctxT_rt = attention("rt", qp_rt, kp_rt, vp_rt)
        rt_bf, _ = out_proj("rt", ctxT_rt, None)
        dump_feat("d_t_r", t_r)
        dump_feat("d_rt", rt_bf)
        dump_feat("d_lfn", lfn)

        # ---------- final: full = rt @ t_r.T, cosine logits ----------
        # row-major full (for row norms): out[q(part), c] = sum_e rt[e,q] t_r[e,c]
        sq_scratch = finals.tile([P, 512], BF16, tag="fsq", name="fsq")
        frow = finals.tile([P, E], BF16, tag="frow", name="frow")
        acc = finals.tile([P, 2], F32, tag="acc2", name="acc_rn")
        for h4 in range(2):
            ps = psA.tile([P, 512], F32, tag="mm", name="ps_fr")
            for ci in range(KO):
                nc.tensor.matmul(ps, rt_bf[:, ci],
                                 t_r[:, ci, h4 * 512:(h4 + 1) * 512],
                                 start=(ci == 0), stop=(ci == KO - 1))
            nc.scalar.activation(frow[:, h4 * 512:(h4 + 1) * 512], ps, AF.Copy)
            nc.scalar.activation(sq_scratch, ps, AF.Square,
                                 accum_out=acc[:, h4:h4 + 1])
        rn = finals.tile([P, 1], F32, tag="rn", name="rn")
        nc.vector.tensor_add(rn, acc[:, 0:1], acc[:, 1:2])
        nc.scalar.sqrt(rn, rn)
        nc.vector.tensor_scalar_max(rn, rn, EPS)
        inv_q = finals.tile([P, 1], F32, tag="invq", name="inv_q")
        nc.vector.reciprocal(inv_q, rn)

        # feat-major fullT (logits lhsT) via PE transpose of full_row
        fullT = finals.tile([P, KO, L], BF16, tag="fullT", name="fullT")
        for cc in range(KO):
            pt = psT.tile([P, P], BF16, tag="tr", name="ps_ftr")
            nc.tensor.transpose(pt, frow[:, cc * P:(cc + 1) * P], ident)
            nc.vector.tensor_copy(fullT[:, cc], pt)

        dump_plain("d_frow", frow)
        lg = finals.tile([P, GRP], F32, tag="lg", name="lg")
        ps = psA.tile([P, 512], F32, tag="mm", name="ps_lg")
        for cc in range(KO):
            nc.tensor.matmul(ps[:, :GRP], fullT[:, cc], lfn[:, cc],
                             start=(cc == 0), stop=(cc == KO - 1))
        nc.vector.tensor_scalar_mul(lg, ps[:, :GRP], inv_q)
        nc.sync.dma_start(out_logits, lg)

    nc.compile()
    return nc


def make_in_maps(local_feat, global_feat, text_feat,
                 w_tl, b_tl, w_tg, b_tg, w_rep, b_rep,
                 pos_local, pos_global, mha_params):
    f32 = np.float32
    bf16 = ml_dtypes.bfloat16
    textT = np.ascontiguousarray(text_feat.T.astype(bf16))
    locT = np.ascontiguousarray(local_feat.T.astype(bf16))
    globT = np.ascontiguousarray(global_feat.T.astype(bf16))
    shared = {
        "x_text": textT, "x_loc": locT, "x_glob": globT,
        "w_tl": np.ascontiguousarray(w_tl.T.astype(bf16)),
        "w_tg": np.ascontiguousarray(w_tg.T.astype(bf16)),
        "w_rep": np.ascontiguousarray(w_rep.T.astype(bf16)),
        "b_tl": b_tl.astype(f32), "b_tg": b_tg.astype(f32),
        "b_rep": b_rep.astype(f32),
        "pos_l": pos_local.astype(f32), "pos_g": pos_global.astype(f32),
    }
    for m, (wi, bi, wo, bo) in mha_params.items():
        shared[f"wq_{m}"] = np.ascontiguousarray(wi[0 * E:1 * E].T.astype(bf16))
        shared[f"wk_{m}"] = np.ascontiguousarray(wi[1 * E:2 * E].T.astype(bf16))
        shared[f"wv_{m}"] = np.ascontiguousarray(wi[2 * E:3 * E].T.astype(bf16))
        shared[f"wo_{m}"] = np.ascontiguousarray(wo.T.astype(bf16))
        shared[f"bq_{m}"] = bi[0 * E:1 * E].astype(f32)
        # V bias folded into output-projection bias: bo_eff = bo + wo @ bv
        shared[f"bo_{m}"] = (bo + wo @ bi[2 * E:3 * E]).astype(f32)

    in_maps = []
    for c in range(NCORES):
        g = c // 2
        m = dict(shared)
        m["x_text_own"] = np.ascontiguousarray(textT[:, c * L:(c + 1) * L])
        m["x_loc_grp"] = np.ascontiguousarray(locT[:, g * GRP:(g + 1) * GRP])
        in_maps.append(m)
    return in_maps


def kernel(local_feat, global_feat, text_feat,
           w_tl, b_tl, w_tg, b_tg, w_rep, b_rep,
           pos_local, pos_global,
           tl_wi, tl_bi, tl_wo, tl_bo,
           tg_wi, tg_bi, tg_wo, tg_bo,
           ff_wi, ff_bi, ff_wo, ff_bo,
           rt_wi, rt_bi, rt_wo, rt_bo,
           n_groups):
    assert int(n_groups) == 4
    if "nc" not in _CACHE:
        _CACHE["nc"] = build_nc()
    nc = _CACHE["nc"]
    mha_params = {
        "tl": (tl_wi, tl_bi, tl_wo, tl_bo),
        "tg": (tg_wi, tg_bi, tg_wo, tg_bo),
        "ff": (ff_wi, ff_bi, ff_wo, ff_bo),
        "rt": (rt_wi, rt_bi, rt_wo, rt_bo),
    }
    in_maps = make_in_maps(np.asarray(local_feat), np.asarray(global_feat),
                           np.asarray(text_feat),
                           np.asarray(w_tl), np.asarray(b_tl),
                           np.asarray(w_tg), np.asarray(b_tg),
                           np.asarray(w_rep), np.asarray(b_rep),
                           np.asarray(pos_local), np.asarray(pos_global),
                           {k: tuple(np.asarray(x) for x in v)
                            for k, v in mha_params.items()})
    res = run_bass_kernel_spmd(nc, in_maps, core_ids=list(range(NCORES)))
    _CACHE["last_results"] = res
    out = np.empty((4, GRP, GRP), dtype=np.float32)
    for c in range(NCORES):
        g, half = c // 2, c % 2
        out[g, half * L:(half + 1) * L, :] = res.results[c]["logits"]
    return out


# revision 56
# speedup vs baseline: 2.6226x; 1.3713x over previous
"""Trainium2 Bass kernel for nn_Model4 (retrieval_knn).

Model: 3 l2-normalized feature streams -> 4 chained MultiheadAttention blocks
-> full = rt @ t_r.T -> per-group cosine logits [4, 256, 256].

Sharding (v2): 8-way row sharding (core c owns rows [128c, 128c+128)) with
REPLICATED K/V projections.  K/V sources for the tl/tg MHAs (local_n+pos,
global_n+pos) derive from inputs, so every core computes full-sequence K/V
locally; only the two intermediate activations that cross MHAs (gt, ff) are
AllGather'd.  That cuts the collective count from 6 (614us) to 2 (135us), and
both gathers overlap with independent compute (tl-MHA during the gt gather,
t_r/final-stage prep during the ff gather).

Bias algebra: K-projection bias drops out of softmax (adds a per-query
constant to every score); V-projection bias is folded into the output
projection bias host-side (bo_eff = bo + wo @ bv).

Layouts: activations feat-major ([chan(part) x chunks, rows(free)]); V
projections row-major ([S(part) x chunks, dv(free)]) so they serve as AV
lhsT directly; attention uses transposed-softmax (no max subtraction; scores
are ~1e-3).  Gathered tensors keep rank-major S order == global row order.
The full-text normalization scale commutes through the t_r GEMM (columns
scaled post-GEMM), so normalized full text is never materialized.
"""
import sys

sys.path.insert(0, "/opt/trn_rl_repo")

import ml_dtypes
import numpy as np

import concourse.bass as bass  # noqa: F401
import concourse.tile as tile
import concourse.mybir as mybir
from concourse import bacc
from concourse.bass_utils import run_bass_kernel_spmd
from concourse.masks import make_identity

E = 1024
P = 128
KO = 8               # feature chunks of 128
L = 128              # rows per core
GRP = 256            # rows per output group
NCORES = 8
F32 = mybir.dt.float32
F32R = mybir.dt.float32r
BF16 = mybir.dt.bfloat16
F8 = mybir.dt.float8e4
DR = mybir.MatmulPerfMode.DoubleRow
AF = mybir.ActivationFunctionType
GROUPS8 = [[0, 1, 2, 3, 4, 5, 6, 7]]
EPS = 1e-8
PIECE = P * KO * L   # 131072 bf16 elements in one packed [128,8,128] piece

DEBUG = False
_CACHE = {}


def build_nc():
    nc = bacc.Bacc("TRN2", target_bir_lowering=False, debug=False,
                   num_devices=NCORES)
    dram = {}

    def din(name, shape, dt=BF16):
        dram[name] = nc.dram_tensor(name, shape, dt, kind="ExternalInput").ap()

    # full feat-major feature streams; glob/loc only feed the K/V
    # projections so they ship as raw fp8 (values are ~N(0,1))
    din("x_glob", [E, E], F8)
    din("x_text", [E, E])
    din("x_loc", [E, E], F8)
    # per-core slices
    din("x_text_own", [E, L])
    din("x_loc_grp", [E, GRP])
    # shared projections, host-transposed to [cin, cout]
    for w in ("w_tl", "w_tg", "w_rep"):
        din(w, [E, E])
    for b in ("b_tl", "b_tg", "b_rep"):
        din(b, [E], F32)
    # host-folded K-projection pos terms: 16 * (wk @ pos)
    din("kpos_tl", [E], F32)
    din("kpos_tg", [E], F32)
    # per-MHA weights, host-transposed to [cin, cout]; K bias dropped,
    # V bias folded into bo host-side.  Q/K/V weights in fp8 (x16 host
    # scaling), output projection in bf16.
    for m in ("tl", "tg", "ff", "rt"):
        for w in ("wq", "wk", "wv"):
            # rt's V path stays bf16: its output has no residual protection,
            # so fp8 V-side noise there would land directly on the logits.
            # rt's K/Q path is fp8 (softmax washes that noise out).
            din(f"{w}_{m}", [E, E], BF16 if (m, w) == ("rt", "wv") else F8)
        din(f"wo_{m}", [E, E])
        din(f"bq_{m}", [E], F32)
        din(f"bo_{m}", [E], F32)

    din("bp_tg", [E], F32)    # packed-gt bias:  bo_eff_tg - c_g
    din("bp_ff", [E], F32)    # packed-ff bias:  bo_eff_ff - c_f
    out_logits = nc.dram_tensor("logits", [L, GRP], F32,
                                kind="ExternalOutput").ap()
    dbg = {}
    if DEBUG:
        for nm, shape, dt in [
                ("d_kvg", [E, E], F8), ("d_textn_own", [E, L], BF16),
                ("d_t_g", [E, L], F8), ("d_qp_tg", [E, L], F8),
                ("d_kp_tg", [E, E], F8), ("d_vp_tg", [E, E], BF16),
                ("d_ctx_tg", [L, E], BF16), ("d_gt", [E, L], F8),
                ("d_gtf", [KO, E, L], F8), ("d_lt", [E, L], F32),
                ("d_ff", [E, L], BF16), ("d_rt", [E, L], BF16),
                ("d_t_r", [E, E], BF16), ("d_frow", [L, E], BF16),
                ("d_lfn", [E, GRP], BF16)]:
            dbg[nm] = nc.dram_tensor(nm, shape, dt,
                                     kind="ExternalOutput").ap()

    from contextlib import ExitStack
    with tile.TileContext(nc) as tc, ExitStack() as ctx:
        def pool(name, bufs, space="SBUF"):
            return ctx.enter_context(
                tc.tile_pool(name=name, bufs=bufs, space=space))

        consts = pool("consts", 1)
        raws = pool("raws", 1)       # one big raw stream buffer (16k)
        rawsm = pool("rawsm", 1)     # small raw slices (3k)
        acts = pool("acts", 1)       # textn_own (2k)
        gath = pool("gath", 1)       # gathered gt/ff (16k + 8k fp8 copy)
        kps = pool("kps", 1)         # K proj (16k)
        vps = pool("vps", 1)         # V proj (16k)
        exps = pool("exps", 1)       # exp scores (8k)
        ctxs = pool("ctxs", 1)       # ctx + ctxT (4k)
        pers = pool("pers", 1)       # t_r (16k) + lfn (4k)
        bcs = pool("bcs", 1)         # broadcast tiles (~10k)
        smalls = pool("smalls", 1)   # inv/nrm rows (~10k)
        finals = pool("finals", 1)   # final-stage tiles (~5k)
        sqs = pool("sqs", 2)         # squared chunks (2k x2)
        pers2 = pool("pers2", 2)     # f32 masters t_g/t_l/lt (12k)
        qps = pool("qps", 2)         # q projections (4k)
        outs_p = pool("outs_p", 2)   # MHA outputs pre-pack (4k)
        weights = pool("weights", 3)  # streamed weights (48k)
        psA = pool("psA", 6, space="PSUM")
        pssum = pool("pssum", 1, space="PSUM")
        psT = pool("psT", 1, space="PSUM")
        dram_p = pool("dram_p", 1, space="DRAM")

        # ---------- constants ----------
        ones_cb = consts.tile([P, 1], BF16)
        nc.vector.memset(ones_cb, 1.0)
        ones_cf32 = consts.tile([P, 1], F32)
        nc.vector.memset(ones_cf32, 1.0)
        ones_cr = consts.tile([P, 1], F32R)
        nc.vector.tensor_copy(ones_cr, ones_cf32)
        ones_rf32 = consts.tile([1, P], F32)
        nc.vector.memset(ones_rf32, 1.0)
        ones_rr = consts.tile([1, P], F32R)
        nc.vector.tensor_copy(ones_rr, ones_rf32)
        ident = consts.tile([P, P], BF16)
        make_identity(nc, ident)

        def load_bias_pp(name):
            t = consts.tile([P, KO], F32, name=f"c_{name}")
            nc.sync.dma_start(t, dram[name].rearrange("(c p) -> p c", p=P))
            return t

        bias_pp = {}
        for nm in ("b_tl", "b_tg", "b_rep", "kpos_tl", "kpos_tg"):
            bias_pp[nm] = load_bias_pp(nm)
        for m in ("tl", "tg", "ff", "rt"):
            bias_pp[f"bq_{m}"] = load_bias_pp(f"bq_{m}")
            bias_pp[f"bo_{m}"] = load_bias_pp(f"bo_{m}")
        bias_pp["bp_tg"] = load_bias_pp("bp_tg")
        bias_pp["bp_ff"] = load_bias_pp("bp_ff")

        # ---------- helpers ----------
        def load_w(name):
            """[1024, 1024] dram -> [128, 8, 1024] (p, cin-chunk, cout)."""
            dt = dram[name].dtype
            t = weights.tile([P, KO, E], dt, tag="w", name=f"w_{name}",
                             padded_shape=[P, KO, E] if dt == F8 else None)
            nc.sync.dma_start(t, dram[name].rearrange("(ko p) c -> p ko c",
                                                      p=P))
            return t

        def norm_inv(raw, n, ncols=None, scale=1.0):
            """Per-free-column scale/l2norm over all 8 chunks of raw
            [128,8,n].  Returns [1, n] f32r."""
            ncols = ncols or n
            nhalf = (ncols + 511) // 512
            nrm = smalls.tile([1, ncols], F32R, tag="nrm", name="nrm",
                              padded_shape=[1, E])
            inv = nrm
            for h in range(nhalf):
                lo = h * 512
                hi = min(ncols, lo + 512)
                pss = pssum.tile([1, min(512, ncols)], F32, tag="cs",
                                 name="ps_cs")
                for ko in range(KO):
                    sq = sqs.tile([P, 512], F32R, tag="sq", name="sq",
                                  padded_shape=[P, 512])
                    nc.scalar.activation(sq[:, :hi - lo], raw[:, ko, lo:hi],
                                         AF.Square)
                    nc.tensor.matmul(pss[:, :hi - lo], ones_cr,
                                     sq[:, :hi - lo], start=(ko == 0),
                                     stop=(ko == KO - 1))
                # sqrt(x / scale^2) = ||x|| / scale  ->  inv = scale/||x||
                nc.scalar.activation(nrm[:, lo:hi], pss[:, :hi - lo],
                                     AF.Sqrt, scale=1.0 / (scale * scale))
            with nc.allow_low_precision(reason="norm reciprocal"):
                nc.vector.reciprocal(inv, nrm)
            return inv


        def bcast_row(row_r, n, dtype, tag, name="bc"):
            """[1, n] f32r -> [128, n] broadcast tile."""
            out = bcs.tile([P, n], dtype, tag=tag, name=name)
            for h in range((n + 511) // 512):
                lo = h * 512
                hi = min(n, lo + 512)
                ps = psA.tile([P, 512], F32, tag="mm", name="ps_bc")
                nc.tensor.matmul(ps[:, :hi - lo], ones_rr, row_r[:, lo:hi],
                                 start=True, stop=True)
                nc.scalar.activation(out[:, lo:hi], ps[:, :hi - lo], AF.Copy)
            return out

        def load_raw(xname, n, big):
            p = raws if big else rawsm
            raw = p.tile([P, KO, n], dram[xname].dtype,
                         tag="raw1024" if big else f"raw{n}",
                         name=f"raw_{xname}",
                         padded_shape=[P, KO, E] if big else None)
            for ko in range(KO):
                nc.sync.dma_start(raw[:, ko],
                                  dram[xname][ko * P:(ko + 1) * P, :])
            return raw

        def normalize(xname, n, out_pool, pos=None, tag=None, big=False,
                      dtype=BF16, scale=1.0):
            """bf16 feat-major [E, n] dram -> scale*l2norm rows, [128,8,n].
            With pos: returns (k_tile with pos added, v_tile without pos) --
            the V path excludes the constant pos component (folded into the
            output-projection bias host-side) so that systematic fp8
            weight-quantization noise has no constant carrier."""
            raw = load_raw(xname, n, big)
            inv = norm_inv(raw, n, scale=scale)
            bc = bcast_row(inv, n, F32R, tag=f"bcn{n}", name=f"bc_{xname}")
            out = out_pool.tile([P, KO, n], dtype, tag=tag or f"nb{n}",
                                name=f"n_{xname}")
            for ko in range(KO):
                nc.vector.tensor_mul(out[:, ko], raw[:, ko], bc)
            if pos is None:
                return out
            outk = out_pool.tile([P, KO, n], dtype, tag=(tag or f"nb{n}") + "k",
                                 name=f"nk_{xname}")
            for ko in range(KO):
                nc.vector.tensor_scalar_add(outk[:, ko], out[:, ko],
                                            pos[:, ko:ko + 1])
            return outk, out

        def gemm_own(w_sb, src_bf, bias, name, residual=None, master=False,
                     out_pool=None, out_dt=BF16, oscale=1.0, in_scale=1.0,
                     perf_mode=None):
            """Own-rows GEMM via fat row-major matmuls (lhsT = feat-major
            src), then PE-transpose back to feat-major [128, 8(co), 128(L)].
            Emits out = ((psum/in_scale) + bias) * oscale (+ residual, which
            must already be in oscale).  With perf_mode=DR, src/w are fp8
            and contraction runs two 128-chunks per matmul."""
            pool_ = out_pool or outs_p
            out_bf = pool_.tile([P, KO, L], out_dt, tag="ob", name=name,
                                padded_shape=[P, KO, L])
            out_f = None
            if master:
                # master stays at TRUE scale; the secondary copy applies
                # oscale (e.g. x32 into fp8).
                out_f = pool_.tile([P, KO, L], F32, tag="of", name=name + "_f")
            row = ctxs.tile([P, E], BF16, tag="grow", name=f"row_{name}")
            pss = [psA.tile([P, 512], F32, tag="mm", name=f"ps_go{h}")
                   for h in range(2)]
            nk = 4 if perf_mode is DR else KO
            for ci in range(nk):
                for h in range(2):
                    if perf_mode is DR:
                        nc.tensor.matmul(pss[h], src_bf[:, 2 * ci:2 * ci + 2],
                                         w_sb[:, 2 * ci:2 * ci + 2,
                                              h * 512:(h + 1) * 512],
                                         start=(ci == 0), stop=(ci == nk - 1),
                                         perf_mode=DR)
                    else:
                        nc.tensor.matmul(pss[h], src_bf[:, ci],
                                         w_sb[:, ci, h * 512:(h + 1) * 512],
                                         start=(ci == 0), stop=(ci == nk - 1))
            for h in range(2):
                nc.scalar.activation(row[:, h * 512:(h + 1) * 512], pss[h],
                                     AF.Copy, scale=1.0 / in_scale)
            for co in range(KO):
                pt = psT.tile([P, P], BF16, tag="tr", name="ps_gt")
                nc.tensor.transpose(pt, row[:, co * P:(co + 1) * P], ident)
                tgt = out_f if master else out_bf
                mscale = 1.0 if master else oscale
                if bias is not None:
                    if mscale != 1.0:
                        nc.vector.tensor_scalar(tgt[:, co], pt,
                                                bias[:, co:co + 1], mscale,
                                                mybir.AluOpType.add,
                                                mybir.AluOpType.mult)
                    else:
                        nc.vector.tensor_scalar_add(tgt[:, co], pt,
                                                    bias[:, co:co + 1])
                    if residual is not None:
                        nc.vector.tensor_add(tgt[:, co], tgt[:, co],
                                             residual[:, co])
                elif residual is not None:
                    nc.vector.tensor_add(tgt[:, co], pt, residual[:, co])
                else:
                    nc.vector.tensor_copy(tgt[:, co], pt)
                if master:
                    nc.scalar.activation(out_bf[:, co], out_f[:, co], AF.Copy,
                                         scale=oscale)
            return out_bf, out_f

        # fp8 kv source accessors (k-tile pairs for DoubleRow):
        # plain [128, 8(ci), 1024(S)] or gathered ci-major
        # [128, 8(ci), 8(rank), 128]
        def src_rhs(src, kt, h4):
            if len(src.shape) == 4:
                return src[:, 2 * kt:2 * kt + 2, h4 * 4:(h4 + 1) * 4, :]
            return src[:, 2 * kt:2 * kt + 2, h4 * 512:(h4 + 1) * 512]

        def src_lhsT(src, kt, s):
            if len(src.shape) == 4:
                return src[:, 2 * kt:2 * kt + 2, s, :]
            return src[:, 2 * kt:2 * kt + 2, s * P:(s + 1) * P]

        def transpose_inv(inv, name):
            """[1, E] f32r row -> [128, 8] per-partition scalars, via a tiny
            DRAM round-trip (strided reload transposes for free)."""
            scratch = dram_p.tile([E], F32, name=name + "_d")
            nc.sync.dma_start(scratch, inv.bitcast(F32))
            t = smalls.tile([P, KO], F32, tag="bcnT", name=name)
            nc.sync.dma_start(t, scratch.rearrange("(a p) -> p a", p=P))
            return t

        def kv_project_raw(m, raw, bc, bcnT, kpos_pp):
            """K/V projections straight from the raw fp8 stream; the
            normalization scale is applied per-column at evacuation and the
            pos contribution to K is a host-folded per-chan constant.
            kp = 16*true (fp8); vp = true (bf16)."""
            wk = load_w(f"wk_{m}")
            kp = kps.tile([P, KO, E], F8, tag="kp", name=f"kp_{m}",
                          padded_shape=[P, KO, E])
            for co in range(KO):
                for h4 in range(2):
                    sl = slice(h4 * 512, (h4 + 1) * 512)
                    ps = psA.tile([P, 512], F32, tag="mm", name="ps_k")
                    for kt in range(4):
                        nc.tensor.matmul(ps,
                                         wk[:, 2 * kt:2 * kt + 2,
                                            co * P:(co + 1) * P],
                                         raw[:, 2 * kt:2 * kt + 2, sl],
                                         start=(kt == 0), stop=(kt == 3),
                                         perf_mode=DR)
                    # two-op evac: psum-side multiply on DVE (gpsimd
                    # cannot touch PSUM), SBUF-side pos add on the idle
                    # Pool engine.
                    nc.vector.tensor_mul(kp[:, co, sl], ps, bc[:, sl])
                    nc.gpsimd.tensor_scalar_add(kp[:, co, sl], kp[:, co, sl],
                                                kpos_pp[:, co:co + 1])
            wv = load_w(f"wv_{m}")
            vp = vps.tile([P, KO, E], BF16, tag="vp", name=f"vp_{m}")
            for sch in range(KO):
                for dh in range(2):
                    sl = slice(dh * 512, (dh + 1) * 512)
                    ps = psA.tile([P, 512], F32, tag="mm", name="ps_v")
                    for kt in range(4):
                        nc.tensor.matmul(ps,
                                         raw[:, 2 * kt:2 * kt + 2,
                                             sch * P:(sch + 1) * P],
                                         wv[:, 2 * kt:2 * kt + 2, sl],
                                         start=(kt == 0), stop=(kt == 3),
                                         perf_mode=DR)
                    nc.vector.tensor_scalar(vp[:, sch, sl], ps,
                                            bcnT[:, sch:sch + 1], 1.0 / 16.0,
                                            mybir.AluOpType.mult,
                                            mybir.AluOpType.mult)
            return kp, vp

        def src_rhs_bf(src, ci, h4):
            if len(src.shape) == 4:
                return src[:, ci, h4 * 4:(h4 + 1) * 4, :]
            return src[:, ci, h4 * 512:(h4 + 1) * 512]

        def src_lhsT_bf(src, ci, s):
            if len(src.shape) == 4:
                return src[:, ci, s, :]
            return src[:, ci, s * P:(s + 1) * P]

        def kv_project(m, src, src_scale=1.0, src_v=None, v_scale=None,
                       v_first=False):
            """K/V projections over all 1024 rows.  fp8 sources run
            DoubleRow (kp comes out x16 fp8); bf16 sources run standard
            matmuls (kp bf16).  vp always at true scale (bf16)."""
            f8 = src.dtype == F8
            gain = src_scale * (16.0 if f8 else 1.0)
            if src_v is None:
                src_v = src
            v8 = src_v.dtype == F8
            vgain = v_scale if v_scale is not None else gain

            def do_k():
                wk = load_w(f"wk_{m}")
                kp = kps.tile([P, KO, E], F8 if f8 else BF16, tag="kp",
                              name=f"kp_{m}", padded_shape=[P, KO, E])
                for co in range(KO):
                    for h4 in range(2):
                        ps = psA.tile([P, 512], F32, tag="mm", name="ps_k")
                        if f8:
                            for kt in range(4):
                                nc.tensor.matmul(ps,
                                                 wk[:, 2 * kt:2 * kt + 2,
                                                    co * P:(co + 1) * P],
                                                 src_rhs(src, kt, h4),
                                                 start=(kt == 0),
                                                 stop=(kt == 3),
                                                 perf_mode=DR)
                        else:
                            for ci in range(KO):
                                nc.tensor.matmul(ps,
                                                 wk[:, ci,
                                                    co * P:(co + 1) * P],
                                                 src_rhs_bf(src, ci, h4),
                                                 start=(ci == 0),
                                                 stop=(ci == KO - 1))
                        sc = (16.0 if f8 else 1.0) / gain
                        nc.scalar.activation(
                            kp[:, co, h4 * 512:(h4 + 1) * 512], ps, AF.Copy,
                            scale=sc)
                return kp

            def do_v():
                wv = load_w(f"wv_{m}")
                vp = vps.tile([P, KO, E], BF16, tag="vp", name=f"vp_{m}")
                for s_ in range(KO):
                    for dh in range(2):
                        ps = psA.tile([P, 512], F32, tag="mm", name="ps_v")
                        if v8:
                            for kt in range(4):
                                nc.tensor.matmul(ps, src_lhsT(src_v, kt, s_),
                                                 wv[:, 2 * kt:2 * kt + 2,
                                                    dh * 512:(dh + 1) * 512],
                                                 start=(kt == 0),
                                                 stop=(kt == 3),
                                                 perf_mode=DR)
                        else:
                            for ci in range(KO):
                                nc.tensor.matmul(ps,
                                                 src_lhsT_bf(src_v, ci, s_),
                                                 wv[:, ci,
                                                    dh * 512:(dh + 1) * 512],
                                                 start=(ci == 0),
                                                 stop=(ci == KO - 1))
                        if vgain == 1.0:
                            nc.vector.tensor_copy(
                                vp[:, s_, dh * 512:(dh + 1) * 512], ps)
                        else:
                            nc.vector.tensor_scalar_mul(
                                vp[:, s_, dh * 512:(dh + 1) * 512], ps,
                                1.0 / vgain)
                return vp

            if v_first:
                vp = do_v()
                kp = do_k()
            else:
                kp = do_k()
                vp = do_v()
            return kp, vp

        def attention(m, qp, kp, vp):
            """-> ctxT [128, 8(ci), 128(L)] bf16 (pre-out-proj context)."""
            f8 = kp.dtype == F8
            expt = exps.tile([P, KO, 512], BF16, tag="exp", name=f"expt_{m}")
            for s in range(KO):
                ps = psA.tile([P, 512], F32, tag="mm", name="ps_sc")
                for h in range(4):
                    if f8:
                        nc.tensor.matmul(
                            ps[:, h * P:(h + 1) * P],
                            kp[:, 2 * h:2 * h + 2, s * P:(s + 1) * P],
                            qp[:, 2 * h:2 * h + 2], start=True, stop=True,
                            perf_mode=DR)
                    else:
                        for dk in range(2):
                            nc.tensor.matmul(
                                ps[:, h * P:(h + 1) * P],
                                kp[:, 2 * h + dk, s * P:(s + 1) * P],
                                qp[:, 2 * h + dk], start=(dk == 0),
                                stop=(dk == 1))
                # fp8 kp and qp both carry x16 -> scores x256
                nc.scalar.activation(expt[:, s], ps, AF.Exp,
                                     scale=0.0625 / 256.0 if f8 else 0.0625)
            pss = pssum.tile([1, 512], F32, tag="cs", name="ps_sm")
            for s in range(KO):
                nc.tensor.matmul(pss, ones_cb, expt[:, s], start=(s == 0),
                                 stop=(s == KO - 1))
            inv = smalls.tile([1, 512], F32R, tag="inv512", name="inv_sm")
            with nc.allow_low_precision(reason="softmax reciprocal"):
                nc.vector.reciprocal(inv, pss)
            bc = bcast_row(inv, 512, BF16, tag="bcs", name=f"bcs_{m}")
            for s in range(KO):
                nc.vector.tensor_mul(expt[:, s], expt[:, s], bc)
            ctx = ctxs.tile([P, E], BF16, tag="ctx", name=f"ctx_{m}")
            for hh in range(2):
                ps = psA.tile([P, 512], F32, tag="mm", name="ps_av")
                for hi in range(2):
                    h = 2 * hh + hi
                    for s in range(KO):
                        nc.tensor.matmul(
                            ps[:, hi * 256:(hi + 1) * 256],
                            expt[:, s, h * P:(h + 1) * P],
                            vp[:, s, h * 256:(h + 1) * 256],
                            start=(s == 0), stop=(s == KO - 1))
                nc.scalar.activation(ctx[:, hh * 512:(hh + 1) * 512], ps,
                                     AF.Copy)
            if DEBUG and m == "tg":
                nc.sync.dma_start(dbg["d_ctx_tg"], ctx)
            ctxT = ctxs.tile([P, KO, L], BF16, tag="ctxT", name=f"ctxT_{m}")
            for ci in range(KO):
                pt = psT.tile([P, P], BF16, tag="tr", name="ps_tr")
                nc.tensor.transpose(pt, ctx[:, ci * P:(ci + 1) * P], ident)
                nc.vector.tensor_copy(ctxT[:, ci], pt)
            return ctxT

        def out_proj(m, ctxT, residual, master=False, out_pool=None,
                     out_dt=BF16, oscale=1.0, bias=None):
            wo = load_w(f"wo_{m}")
            return gemm_own(wo, ctxT, bias or bias_pp[f"bo_{m}"], f"o_{m}",
                            residual=residual, master=master,
                            out_pool=out_pool, out_dt=out_dt, oscale=oscale)

        def dump_feat(nm, t):
            if DEBUG:
                nc.sync.dma_start(
                    dbg[nm].rearrange("(ko p) r -> p ko r", p=P), t)

        def dump_plain(nm, t):
            if DEBUG:
                nc.sync.dma_start(dbg[nm], t)

        def pack_piece(inbuf, sb_tile):
            # NB: collective buffers must be bf16/f32 -- f32r payloads get
            # mantissa-squashed by the collective transport in this runtime.
            nc.sync.dma_start(
                inbuf.rearrange("(p a b) -> p a b", p=P, a=KO), sb_tile)

        def allgather(inbuf, outbuf):
            nc.gpsimd.collective_compute(
                "AllGather", mybir.AluOpType.bypass,
                replica_groups=GROUPS8,
                ins=[inbuf.opt()], outs=[outbuf.opt()])

        def unpack_gather(outbuf, name):
            # ci-major gathered layout [128, 8(ci), 8(rank), 128] so that
            # DoubleRow k-tile slices are contiguous in dim 1.
            t = gath.tile([P, KO, KO, L], outbuf.dtype, tag="gf", name=name)
            for r in range(KO):
                nc.sync.dma_start(
                    t[:, :, r],
                    outbuf[r].rearrange("(p a b) -> p a b", p=P, a=KO))
            return t

        # ---------- stage 0 ----------
        textn_own = normalize("x_text_own", L, acts, tag="nto")
        # glob/loc raw fp8 streams feed K/V projections directly; the l2
        # normalization scale is applied at psum evacuation.
        raw_g = load_raw("x_glob", E, big=True)
        inv_g = norm_inv(raw_g, E)
        bc_g = bcast_row(inv_g, E, BF16, tag="bcn1024", name="bc_g")
        bcnT_g = transpose_inv(inv_g, "bcnT_g")

        # ---------- tg path ----------
        w_tg = load_w("w_tg")
        t_g_f8, t_g32 = gemm_own(w_tg, textn_own, bias_pp["b_tg"], "t_g",
                                 master=True, out_pool=pers2, out_dt=F8,
                                 oscale=32.0)
        wq_tg = load_w("wq_tg")
        qp_tg, _ = gemm_own(wq_tg, t_g_f8, bias_pp["bq_tg"], "qp_tg",
                            out_pool=qps, out_dt=F8, oscale=16.0,
                            in_scale=512.0, perf_mode=DR)
        kp_tg, vp_tg = kv_project_raw("tg", raw_g, bc_g, bcnT_g,
                                      bias_pp["kpos_tg"])
        ctxT_tg = attention("tg", qp_tg, kp_tg, vp_tg)
        gt_bf, _ = out_proj("tg", ctxT_tg, t_g32, master=True, out_dt=F8,
                            oscale=32.0, bias=bias_pp["bp_tg"])
        dump_feat("d_textn_own", textn_own)
        dump_feat("d_t_g", t_g_f8)
        dump_feat("d_qp_tg", qp_tg)
        dump_feat("d_kp_tg", kp_tg)
        dump_feat("d_vp_tg", vp_tg)
        dump_feat("d_gt", gt_bf)

        in1 = dram_p.tile([PIECE], F8, name="in1")
        out1 = dram_p.tile([KO, PIECE], F8, name="out1")
        pack_piece(in1, gt_bf)
        allgather(in1, out1)

        # ---------- tl path (overlaps gt gather) ----------
        raw_l = load_raw("x_loc", E, big=True)
        inv_l = norm_inv(raw_l, E)
        bc_l = bcast_row(inv_l, E, BF16, tag="bcn1024", name="bc_l")
        bcnT_l = transpose_inv(inv_l, "bcnT_l")
        w_tl = load_w("w_tl")
        t_l_f8, t_l32 = gemm_own(w_tl, textn_own, bias_pp["b_tl"], "t_l",
                                 master=True, out_pool=pers2, out_dt=F8,
                                 oscale=32.0)
        wq_tl = load_w("wq_tl")
        qp_tl, _ = gemm_own(wq_tl, t_l_f8, bias_pp["bq_tl"], "qp_tl",
                            out_pool=qps, out_dt=F8, oscale=16.0,
                            in_scale=512.0, perf_mode=DR)
        kp_tl, vp_tl = kv_project_raw("tl", raw_l, bc_l, bcnT_l,
                                      bias_pp["kpos_tl"])
        ctxT_tl = attention("tl", qp_tl, kp_tl, vp_tl)
        lt_f8, lt32 = out_proj("tl", ctxT_tl, t_l32, master=True,
                               out_pool=pers2, out_dt=F8, oscale=32.0)
        wq_ff = load_w("wq_ff")
        qp_ff, _ = gemm_own(wq_ff, lt_f8, bias_pp["bq_ff"], "qp_ff",
                            out_pool=qps, out_dt=F8, oscale=16.0,
                            in_scale=512.0, perf_mode=DR)
        # full-text norm scale (for t_r in the next window); the normalized
        # text itself is never materialized -- the scale commutes through
        # the t_r GEMM.
        raw_text = load_raw("x_text", E, big=True)
        inv_text = norm_inv(raw_text, E)
        bc_text = bcast_row(inv_text, E, F32R, tag="bct", name="bc_text")

        # ---------- ff MHA ----------
        gt_full = unpack_gather(out1, "gt_full")
        if DEBUG:
            for r in range(KO):
                nc.sync.dma_start(
                    dbg["d_gtf"][r].rearrange("(ko p) l -> p ko l", p=P),
                    gt_full[:, :, r])
        kp_ff, vp_ff = kv_project("ff", gt_full, 32.0)
        ctxT_ff = attention("ff", qp_ff, kp_ff, vp_ff)
        ff_bf, _ = out_proj("ff", ctxT_ff, lt32, bias=bias_pp["bp_ff"])
        dump_feat("d_lt", lt32)
        dump_feat("d_ff", ff_bf)

        in2 = dram_p.tile([PIECE], BF16, name="in2")
        out2 = dram_p.tile([KO, PIECE], BF16, name="out2")
        pack_piece(in2, ff_bf)
        allgather(in2, out2)

        # ---------- window 2 (overlaps ff gather): t_r + final prep ----------
        w_rep = load_w("w_rep")
        t_r = pers.tile([P, KO, E], BF16, name="t_r")
        for co in range(KO):
            for h4 in range(2):
                ps = psA.tile([P, 512], F32, tag="mm", name="ps_tr2")
                for ci in range(KO):
                    nc.tensor.matmul(ps, w_rep[:, ci, co * P:(co + 1) * P],
                                     raw_text[:, ci, h4 * 512:(h4 + 1) * 512],
                                     start=(ci == 0), stop=(ci == KO - 1))
                sl = t_r[:, co, h4 * 512:(h4 + 1) * 512]
                nc.vector.tensor_mul(sl, ps, bc_text[:, h4 * 512:(h4 + 1) * 512])
                nc.vector.tensor_scalar_add(sl, sl,
                                            bias_pp["b_rep"][:, co:co + 1])
        t_r_own, _ = gemm_own(w_rep, textn_own, bias_pp["b_rep"], "t_r_own",
                              out_dt=F8, oscale=32.0)
        wq_rt = load_w("wq_rt")
        qp_rt, _ = gemm_own(wq_rt, t_r_own, bias_pp["bq_rt"], "qp_rt",
                            out_pool=qps, out_dt=F8, oscale=16.0,
                            in_scale=512.0, perf_mode=DR)
        lfn = normalize("x_loc_grp", GRP, pers, tag="lfn", big=True)

        # ---------- rt MHA ----------
        ff_full = unpack_gather(out2, "ff_full")
        # fp8 x32 copy for the K/Q path (softmax washes fp8 noise); the
        # V path keeps the bf16 original.  Converted per rank chunk so it
        # pipelines with the unpack DMAs; V projection is emitted first so
        # the PE starts on bf16 V work while the conversion drains.
        ff_f8 = gath.tile([P, KO, KO, L], F8, tag="gf8", name="ff_f8")
        for r in range(KO):
            nc.gpsimd.tensor_scalar_mul(ff_f8[:, :, r], ff_full[:, :, r],
                                        32.0)
        kp_rt, vp_rt = kv_project("rt", ff_f8, 32.0, src_v=ff_full,
                                  v_scale=1.0, v_first=True)
        ctxT_rt = attention("rt", qp_rt, kp_rt, vp_rt)
        rt_bf, _ = out_proj("rt", ctxT_rt, None)
        dump_feat("d_t_r", t_r)
        dump_feat("d_rt", rt_bf)
        dump_feat("d_lfn", lfn)

        # ---------- final: full = rt @ t_r.T, cosine logits ----------
        # row-major full (for row norms): out[q(part), c] = sum_e rt[e,q] t_r[e,c]
        sq_scratch = finals.tile([P, 512], BF16, tag="fsq", name="fsq")
        frow = finals.tile([P, E], BF16, tag="frow", name="frow")
        acc = finals.tile([P, 2], F32, tag="acc2", name="acc_rn")
        for h4 in range(2):
            ps = psA.tile([P, 512], F32, tag="mm", name="ps_fr")
            for ci in range(KO):
                nc.tensor.matmul(ps, rt_bf[:, ci],
                                 t_r[:, ci, h4 * 512:(h4 + 1) * 512],
                                 start=(ci == 0), stop=(ci == KO - 1))
            nc.scalar.activation(frow[:, h4 * 512:(h4 + 1) * 512], ps, AF.Copy)
            nc.scalar.activation(sq_scratch, ps, AF.Square,
                                 accum_out=acc[:, h4:h4 + 1])
        rn = finals.tile([P, 1], F32, tag="rn", name="rn")
        nc.vector.tensor_add(rn, acc[:, 0:1], acc[:, 1:2])
        nc.scalar.sqrt(rn, rn)
        nc.vector.tensor_scalar_max(rn, rn, EPS)
        inv_q = finals.tile([P, 1], F32, tag="invq", name="inv_q")
        nc.vector.reciprocal(inv_q, rn)

        # feat-major fullT (logits lhsT) via PE transpose of full_row
        fullT = finals.tile([P, KO, L], BF16, tag="fullT", name="fullT")
        for cc in range(KO):
            pt = psT.tile([P, P], BF16, tag="tr", name="ps_ftr")
            nc.tensor.transpose(pt, frow[:, cc * P:(cc + 1) * P], ident)
            nc.vector.tensor_copy(fullT[:, cc], pt)

        dump_plain("d_frow", frow)
        lg = finals.tile([P, GRP], F32, tag="lg", name="lg")
        ps = psA.tile([P, 512], F32, tag="mm", name="ps_lg")
        for cc in range(KO):
            nc.tensor.matmul(ps[:, :GRP], fullT[:, cc], lfn[:, cc],
                             start=(cc == 0), stop=(cc == KO - 1))
        nc.vector.tensor_scalar_mul(lg, ps[:, :GRP], inv_q)
        nc.sync.dma_start(out_logits, lg)

    nc.compile()
    return nc


def make_in_maps(local_feat, global_feat, text_feat,
                 w_tl, b_tl, w_tg, b_tg, w_rep, b_rep,
                 pos_local, pos_global, mha_params):
    f32 = np.float32
    bf16 = ml_dtypes.bfloat16
    f8 = ml_dtypes.float8_e4m3
    textT = np.ascontiguousarray(text_feat.T.astype(bf16))
    locT = np.ascontiguousarray(local_feat.T.astype(bf16))
    shared = {
        "x_text": textT,
        "x_loc": np.ascontiguousarray(local_feat.T.astype(f8)),
        "x_glob": np.ascontiguousarray(global_feat.T.astype(f8)),
        "w_tl": np.ascontiguousarray(w_tl.T.astype(bf16)),
        "w_tg": np.ascontiguousarray(w_tg.T.astype(bf16)),
        "w_rep": np.ascontiguousarray(w_rep.T.astype(bf16)),
        "b_tl": b_tl.astype(f32), "b_tg": b_tg.astype(f32),
        "b_rep": b_rep.astype(f32),
    }
    wv_f, wo_f, bo_eff = {}, {}, {}
    for m, (wi, bi, wo, bo) in mha_params.items():
        # q/k/v weights in fp8 (x16 into the e4m3 sweet spot), except the
        # precision-critical rt block which stays bf16
        shared[f"wq_{m}"] = np.ascontiguousarray(
            (16.0 * wi[0 * E:1 * E].T).astype(f8))
        shared[f"wk_{m}"] = np.ascontiguousarray(
            (16.0 * wi[1 * E:2 * E].T).astype(f8))
        if m == "rt":
            shared[f"wv_{m}"] = np.ascontiguousarray(
                wi[2 * E:3 * E].T.astype(bf16))
        else:
            shared[f"wv_{m}"] = np.ascontiguousarray(
                (16.0 * wi[2 * E:3 * E].T).astype(f8))
        shared[f"wo_{m}"] = np.ascontiguousarray(wo.T.astype(bf16))
        shared[f"bq_{m}"] = bi[0 * E:1 * E].astype(f32)
        wv_f[m], wo_f[m] = wi[2 * E:3 * E], wo
        # V bias folded into output-projection bias: bo_eff = bo + wo @ bv
        bo_eff[m] = bo + wo @ bi[2 * E:3 * E]
    # The V projections run on pos-free / mean-shifted sources; each removed
    # constant c contributes wo @ (wv @ c) to the block's output bias:
    #   tl/tg: V source excludes pos_local/pos_global
    #   ff:    consumes gt' = gt - c_g   (c_g ~ mean over rows of gt)
    #   rt:    consumes ff' = ff - c_f
    bo_eff["tl"] = bo_eff["tl"] + wo_f["tl"] @ (wv_f["tl"] @ pos_local)
    bo_eff["tg"] = bo_eff["tg"] + wo_f["tg"] @ (wv_f["tg"] @ pos_global)
    c_g = bo_eff["tg"] + b_tg
    bo_eff["ff"] = bo_eff["ff"] + wo_f["ff"] @ (wv_f["ff"] @ c_g)
    c_f = bo_eff["ff"] + bo_eff["tl"] + b_tl
    bo_eff["rt"] = bo_eff["rt"] + wo_f["rt"] @ (wv_f["rt"] @ c_f)
    for m in mha_params:
        shared[f"bo_{m}"] = bo_eff[m].astype(f32)
    shared["bp_tg"] = (bo_eff["tg"] - c_g).astype(f32)
    shared["bp_ff"] = (bo_eff["ff"] - c_f).astype(f32)
    # host-folded K-projection pos terms (x16 to match fp8 kp scaling)
    shared["kpos_tl"] = (16.0 * (mha_params["tl"][0][E:2 * E] @
                                 pos_local)).astype(f32)
    shared["kpos_tg"] = (16.0 * (mha_params["tg"][0][E:2 * E] @
                                 pos_global)).astype(f32)

    in_maps = []
    for c in range(NCORES):
        g = c // 2
        m = dict(shared)
        m["x_text_own"] = np.ascontiguousarray(textT[:, c * L:(c + 1) * L])
        m["x_loc_grp"] = np.ascontiguousarray(locT[:, g * GRP:(g + 1) * GRP])
        in_maps.append(m)
    return in_maps


def kernel(local_feat, global_feat, text_feat,
           w_tl, b_tl, w_tg, b_tg, w_rep, b_rep,
           pos_local, pos_global,
           tl_wi, tl_bi, tl_wo, tl_bo,
           tg_wi, tg_bi, tg_wo, tg_bo,
           ff_wi, ff_bi, ff_wo, ff_bo,
           rt_wi, rt_bi, rt_wo, rt_bo,
           n_groups):
    assert int(n_groups) == 4
    if "nc" not in _CACHE:
        _CACHE["nc"] = build_nc()
    nc = _CACHE["nc"]
    mha_params = {
        "tl": (tl_wi, tl_bi, tl_wo, tl_bo),
        "tg": (tg_wi, tg_bi, tg_wo, tg_bo),
        "ff": (ff_wi, ff_bi, ff_wo, ff_bo),
        "rt": (rt_wi, rt_bi, rt_wo, rt_bo),
    }
    in_maps = make_in_maps(np.asarray(local_feat), np.asarray(global_feat),
                           np.asarray(text_feat),
                           np.asarray(w_tl), np.asarray(b_tl),
                           np.asarray(w_tg), np.asarray(b_tg),
                           np.asarray(w_rep), np.asarray(b_rep),
                           np.asarray(pos_local), np.asarray(pos_global),
                           {k: tuple(np.asarray(x) for x in v)
                            for k, v in mha_params.items()})
    res = run_bass_kernel_spmd(nc, in_maps, core_ids=list(range(NCORES)))
    _CACHE["last_results"] = res
    out = np.empty((4, GRP, GRP), dtype=np.float32)
    for c in range(NCORES):
        g, half = c // 2, c % 2
        out[g, half * L:(half + 1) * L, :] = res.results[c]["logits"]
    return out


# revision 64
# speedup vs baseline: 2.6645x; 1.0160x over previous
"""Trainium2 Bass kernel for nn_Model4 (retrieval_knn).

Model: 3 l2-normalized feature streams -> 4 chained MultiheadAttention blocks
-> full = rt @ t_r.T -> per-group cosine logits [4, 256, 256].

Sharding (v2): 8-way row sharding (core c owns rows [128c, 128c+128)) with
REPLICATED K/V projections.  K/V sources for the tl/tg MHAs (local_n+pos,
global_n+pos) derive from inputs, so every core computes full-sequence K/V
locally; only the two intermediate activations that cross MHAs (gt, ff) are
AllGather'd.  That cuts the collective count from 6 (614us) to 2 (135us), and
both gathers overlap with independent compute (tl-MHA during the gt gather,
t_r/final-stage prep during the ff gather).

Bias algebra: K-projection bias drops out of softmax (adds a per-query
constant to every score); V-projection bias is folded into the output
projection bias host-side (bo_eff = bo + wo @ bv).

Layouts: activations feat-major ([chan(part) x chunks, rows(free)]); V
projections row-major ([S(part) x chunks, dv(free)]) so they serve as AV
lhsT directly; attention uses transposed-softmax (no max subtraction; scores
are ~1e-3).  Gathered tensors keep rank-major S order == global row order.
The full-text normalization scale commutes through the t_r GEMM (columns
scaled post-GEMM), so normalized full text is never materialized.
"""
import sys

sys.path.insert(0, "/opt/trn_rl_repo")

import ml_dtypes
import numpy as np

import concourse.bass as bass  # noqa: F401
import concourse.tile as tile
import concourse.mybir as mybir
from concourse import bacc
from concourse.bass_utils import run_bass_kernel_spmd
from concourse.masks import make_identity

E = 1024
P = 128
KO = 8               # feature chunks of 128
L = 128              # rows per core
GRP = 256            # rows per output group
NCORES = 8
F32 = mybir.dt.float32
F32R = mybir.dt.float32r
BF16 = mybir.dt.bfloat16
F8 = mybir.dt.float8e4
DR = mybir.MatmulPerfMode.DoubleRow
AF = mybir.ActivationFunctionType
GROUPS8 = [[0, 1, 2, 3, 4, 5, 6, 7]]
EPS = 1e-8
PIECE = P * KO * L   # 131072 bf16 elements in one packed [128,8,128] piece

DEBUG = False
_CACHE = {}


def build_nc():
    nc = bacc.Bacc("TRN2", target_bir_lowering=False, debug=False,
                   num_devices=NCORES)
    dram = {}

    def din(name, shape, dt=BF16):
        dram[name] = nc.dram_tensor(name, shape, dt, kind="ExternalInput").ap()

    # full feat-major feature streams; glob/loc only feed the K/V
    # projections so they ship as raw fp8 (values are ~N(0,1))
    din("x_glob", [E, E], F8)
    din("x_text", [E, E])
    din("x_loc", [E, E], F8)
    # per-core slices
    din("x_text_own", [E, L])
    din("x_loc_grp", [E, GRP])
    # shared projections, host-transposed to [cin, cout]
    for w in ("w_tl", "w_tg", "w_rep"):
        din(w, [E, E])
    for b in ("b_tl", "b_tg", "b_rep"):
        din(b, [E], F32)
    # host-folded K-projection pos terms: 16 * (wk @ pos)
    din("kpos_tl", [E], F32)
    din("kpos_tg", [E], F32)
    # per-MHA weights, host-transposed to [cin, cout]; K bias dropped,
    # V bias folded into bo host-side.  Q/K/V weights in fp8 (x16 host
    # scaling), output projection in bf16.
    for m in ("tl", "tg", "ff", "rt"):
        for w in ("wq", "wk", "wv"):
            # rt's V path stays bf16: its output has no residual protection,
            # so fp8 V-side noise there would land directly on the logits.
            # rt's K/Q path is fp8 (softmax washes that noise out).
            din(f"{w}_{m}", [E, E], BF16 if (m, w) == ("rt", "wv") else F8)
        din(f"wo_{m}", [E, E])
        din(f"bq_{m}", [E], F32)
        din(f"bo_{m}", [E], F32)

    din("bp_tg", [E], F32)    # packed-gt bias:  bo_eff_tg - c_g
    din("bp_ff", [E], F32)    # packed-ff bias:  bo_eff_ff - c_f
    out_logits = nc.dram_tensor("logits", [L, GRP], F32,
                                kind="ExternalOutput").ap()
    dbg = {}
    if DEBUG:
        for nm, shape, dt in [
                ("d_kvg", [E, E], F8), ("d_textn_own", [E, L], BF16),
                ("d_t_g", [E, L], F8), ("d_qp_tg", [E, L], F8),
                ("d_kp_tg", [E, E], F8), ("d_vp_tg", [E, E], BF16),
                ("d_ctx_tg", [L, E], BF16), ("d_gt", [E, L], F8),
                ("d_gtf", [KO, E, L], F8), ("d_lt", [E, L], F32),
                ("d_ff", [E, L], BF16), ("d_rt", [E, L], BF16),
                ("d_t_r", [E, E], BF16), ("d_frow", [L, E], BF16),
                ("d_lfn", [E, GRP], BF16)]:
            dbg[nm] = nc.dram_tensor(nm, shape, dt,
                                     kind="ExternalOutput").ap()

    from contextlib import ExitStack
    with tile.TileContext(nc) as tc, ExitStack() as ctx:
        def pool(name, bufs, space="SBUF"):
            return ctx.enter_context(
                tc.tile_pool(name=name, bufs=bufs, space=space))

        consts = pool("consts", 1)
        raws = pool("raws", 1)       # one big raw stream buffer (16k)
        rawsm = pool("rawsm", 1)     # small raw slices (3k)
        acts = pool("acts", 1)       # textn_own (2k)
        gath = pool("gath", 1)       # gathered gt/ff (16k + 8k fp8 copy)
        kps = pool("kps", 1)         # K proj (16k)
        vps = pool("vps", 1)         # V proj (16k)
        exps = pool("exps", 1)       # exp scores (8k)
        ctxs = pool("ctxs", 1)       # ctx + ctxT (4k)
        pers = pool("pers", 1)       # t_r (16k) + lfn (4k)
        bcs = pool("bcs", 1)         # broadcast tiles (~10k)
        smalls = pool("smalls", 1)   # inv/nrm rows (~10k)
        finals = pool("finals", 1)   # final-stage tiles (~5k)
        sqs = pool("sqs", 2)         # squared chunks (2k x2)
        pers2 = pool("pers2", 2)     # f32 masters t_g/t_l/lt (12k)
        qps = pool("qps", 2)         # q projections (4k)
        outs_p = pool("outs_p", 2)   # MHA outputs pre-pack (4k)
        weights = pool("weights", 3)  # streamed weights (48k)
        psA = pool("psA", 6, space="PSUM")
        pssum = pool("pssum", 1, space="PSUM")
        psT = pool("psT", 1, space="PSUM")
        dram_p = pool("dram_p", 1, space="DRAM")

        # ---------- constants ----------
        ones_cb = consts.tile([P, 1], BF16)
        nc.vector.memset(ones_cb, 1.0)
        ones_cf32 = consts.tile([P, 1], F32)
        nc.vector.memset(ones_cf32, 1.0)
        ones_cr = consts.tile([P, 1], F32R)
        nc.vector.tensor_copy(ones_cr, ones_cf32)
        ones_rf32 = consts.tile([1, P], F32)
        nc.vector.memset(ones_rf32, 1.0)
        ones_rr = consts.tile([1, P], F32R)
        nc.vector.tensor_copy(ones_rr, ones_rf32)
        ident = consts.tile([P, P], BF16)
        make_identity(nc, ident)

        def load_bias_pp(name):
            t = consts.tile([P, KO], F32, name=f"c_{name}")
            nc.sync.dma_start(t, dram[name].rearrange("(c p) -> p c", p=P))
            return t

        bias_pp = {}

        def load_biases():
            for nm in ("b_tl", "b_tg", "b_rep", "kpos_tl", "kpos_tg"):
                bias_pp[nm] = load_bias_pp(nm)
            for m in ("tl", "tg", "ff", "rt"):
                bias_pp[f"bq_{m}"] = load_bias_pp(f"bq_{m}")
                bias_pp[f"bo_{m}"] = load_bias_pp(f"bo_{m}")
            bias_pp["bp_tg"] = load_bias_pp("bp_tg")
            bias_pp["bp_ff"] = load_bias_pp("bp_ff")

        # ---------- helpers ----------
        def load_w(name):
            """[1024, 1024] dram -> [128, 8, 1024] (p, cin-chunk, cout)."""
            dt = dram[name].dtype
            t = weights.tile([P, KO, E], dt, tag="w", name=f"w_{name}",
                             padded_shape=[P, KO, E] if dt == F8 else None)
            nc.sync.dma_start(t, dram[name].rearrange("(ko p) c -> p ko c",
                                                      p=P))
            return t

        def norm_inv(raw, n, ncols=None, scale=1.0):
            """Per-free-column scale/l2norm over all 8 chunks of raw
            [128,8,n].  Returns [1, n] f32r."""
            ncols = ncols or n
            nhalf = (ncols + 511) // 512
            nrm = smalls.tile([1, ncols], F32R, tag="nrm", name="nrm",
                              padded_shape=[1, E])
            inv = nrm
            for h in range(nhalf):
                lo = h * 512
                hi = min(ncols, lo + 512)
                pss = pssum.tile([1, min(512, ncols)], F32, tag="cs",
                                 name="ps_cs")
                for ko in range(KO):
                    sq = sqs.tile([P, 512], F32R, tag="sq", name="sq",
                                  padded_shape=[P, 512])
                    nc.scalar.activation(sq[:, :hi - lo], raw[:, ko, lo:hi],
                                         AF.Square)
                    nc.tensor.matmul(pss[:, :hi - lo], ones_cr,
                                     sq[:, :hi - lo], start=(ko == 0),
                                     stop=(ko == KO - 1))
                # sqrt(x / scale^2) = ||x|| / scale  ->  inv = scale/||x||
                nc.scalar.activation(nrm[:, lo:hi], pss[:, :hi - lo],
                                     AF.Sqrt, scale=1.0 / (scale * scale))
            with nc.allow_low_precision(reason="norm reciprocal"):
                nc.vector.reciprocal(inv, nrm)
            return inv


        def bcast_row(row_r, n, dtype, tag, name="bc"):
            """[1, n] f32r -> [128, n] broadcast tile."""
            out = bcs.tile([P, n], dtype, tag=tag, name=name)
            for h in range((n + 511) // 512):
                lo = h * 512
                hi = min(n, lo + 512)
                ps = psA.tile([P, 512], F32, tag="mm", name="ps_bc")
                nc.tensor.matmul(ps[:, :hi - lo], ones_rr, row_r[:, lo:hi],
                                 start=True, stop=True)
                nc.scalar.activation(out[:, lo:hi], ps[:, :hi - lo], AF.Copy)
            return out

        def load_raw(xname, n, big):
            p = raws if big else rawsm
            raw = p.tile([P, KO, n], dram[xname].dtype,
                         tag="raw1024" if big else f"raw{n}",
                         name=f"raw_{xname}",
                         padded_shape=[P, KO, E] if big else None)
            for ko in range(KO):
                nc.sync.dma_start(raw[:, ko],
                                  dram[xname][ko * P:(ko + 1) * P, :])
            return raw

        def normalize(xname, n, out_pool, pos=None, tag=None, big=False,
                      dtype=BF16, scale=1.0, raw=None):
            """bf16 feat-major [E, n] dram -> scale*l2norm rows, [128,8,n].
            With pos: returns (k_tile with pos added, v_tile without pos) --
            the V path excludes the constant pos component (folded into the
            output-projection bias host-side) so that systematic fp8
            weight-quantization noise has no constant carrier."""
            if raw is None:
                raw = load_raw(xname, n, big)
            inv = norm_inv(raw, n, scale=scale)
            bc = bcast_row(inv, n, F32R, tag=f"bcn{n}", name=f"bc_{xname}")
            out = out_pool.tile([P, KO, n], dtype, tag=tag or f"nb{n}",
                                name=f"n_{xname}")
            for ko in range(KO):
                nc.vector.tensor_mul(out[:, ko], raw[:, ko], bc)
            if pos is None:
                return out
            outk = out_pool.tile([P, KO, n], dtype, tag=(tag or f"nb{n}") + "k",
                                 name=f"nk_{xname}")
            for ko in range(KO):
                nc.vector.tensor_scalar_add(outk[:, ko], out[:, ko],
                                            pos[:, ko:ko + 1])
            return outk, out

        def gemm_own(w_sb, src_bf, bias, name, residual=None, master=False,
                     out_pool=None, out_dt=BF16, oscale=1.0, in_scale=1.0,
                     perf_mode=None):
            """Own-rows GEMM via fat row-major matmuls (lhsT = feat-major
            src), then PE-transpose back to feat-major [128, 8(co), 128(L)].
            Emits out = ((psum/in_scale) + bias) * oscale (+ residual, which
            must already be in oscale).  With perf_mode=DR, src/w are fp8
            and contraction runs two 128-chunks per matmul."""
            pool_ = out_pool or outs_p
            out_bf = pool_.tile([P, KO, L], out_dt, tag="ob", name=name,
                                padded_shape=[P, KO, L])
            out_f = None
            if master:
                # master stays at TRUE scale; the secondary copy applies
                # oscale (e.g. x32 into fp8).
                out_f = pool_.tile([P, KO, L], F32, tag="of", name=name + "_f")
            row = ctxs.tile([P, E], BF16, tag="grow", name=f"row_{name}")
            pss = [psA.tile([P, 512], F32, tag="mm", name=f"ps_go{h}")
                   for h in range(2)]
            nk = 4 if perf_mode is DR else KO
            for ci in range(nk):
                for h in range(2):
                    if perf_mode is DR:
                        nc.tensor.matmul(pss[h], src_bf[:, 2 * ci:2 * ci + 2],
                                         w_sb[:, 2 * ci:2 * ci + 2,
                                              h * 512:(h + 1) * 512],
                                         start=(ci == 0), stop=(ci == nk - 1),
                                         perf_mode=DR)
                    else:
                        nc.tensor.matmul(pss[h], src_bf[:, ci],
                                         w_sb[:, ci, h * 512:(h + 1) * 512],
                                         start=(ci == 0), stop=(ci == nk - 1))
            for h in range(2):
                nc.scalar.activation(row[:, h * 512:(h + 1) * 512], pss[h],
                                     AF.Copy, scale=1.0 / in_scale)
            for co in range(KO):
                pt = psT.tile([P, P], BF16, tag="tr", name="ps_gt")
                nc.tensor.transpose(pt, row[:, co * P:(co + 1) * P], ident)
                tgt = out_f if master else out_bf
                mscale = 1.0 if master else oscale
                if bias is not None:
                    if mscale != 1.0:
                        nc.vector.tensor_scalar(tgt[:, co], pt,
                                                bias[:, co:co + 1], mscale,
                                                mybir.AluOpType.add,
                                                mybir.AluOpType.mult)
                    else:
                        nc.vector.tensor_scalar_add(tgt[:, co], pt,
                                                    bias[:, co:co + 1])
                    if residual is not None:
                        nc.vector.tensor_add(tgt[:, co], tgt[:, co],
                                             residual[:, co])
                elif residual is not None:
                    nc.vector.tensor_add(tgt[:, co], pt, residual[:, co])
                else:
                    nc.vector.tensor_copy(tgt[:, co], pt)
                if master:
                    nc.scalar.activation(out_bf[:, co], out_f[:, co], AF.Copy,
                                         scale=oscale)
            return out_bf, out_f

        # fp8 kv source accessors (k-tile pairs for DoubleRow):
        # plain [128, 8(ci), 1024(S)] or gathered ci-major
        # [128, 8(ci), 8(rank), 128]
        def src_rhs(src, kt, h4):
            if len(src.shape) == 4:
                return src[:, 2 * kt:2 * kt + 2, h4 * 4:(h4 + 1) * 4, :]
            return src[:, 2 * kt:2 * kt + 2, h4 * 512:(h4 + 1) * 512]

        def src_lhsT(src, kt, s):
            if len(src.shape) == 4:
                return src[:, 2 * kt:2 * kt + 2, s, :]
            return src[:, 2 * kt:2 * kt + 2, s * P:(s + 1) * P]

        def transpose_inv(inv, name):
            """[1, E] f32r row -> [128, 8] per-partition scalars, via a tiny
            DRAM round-trip (strided reload transposes for free)."""
            scratch = dram_p.tile([E], F32, name=name + "_d")
            nc.sync.dma_start(scratch, inv.bitcast(F32))
            t = smalls.tile([P, KO], F32, tag="bcnT", name=name)
            nc.sync.dma_start(t, scratch.rearrange("(a p) -> p a", p=P))
            return t

        def kv_project_raw(m, raw, bc, bcnT, kpos_pp):
            """K/V projections straight from the raw fp8 stream; the
            normalization scale is applied per-column at evacuation and the
            pos contribution to K is a host-folded per-chan constant.
            kp = 16*true (fp8); vp = true (bf16)."""
            wk = load_w(f"wk_{m}")
            kp = kps.tile([P, KO, E], F8, tag="kp", name=f"kp_{m}",
                          padded_shape=[P, KO, E])
            for co in range(KO):
                for h4 in range(2):
                    sl = slice(h4 * 512, (h4 + 1) * 512)
                    ps = psA.tile([P, 512], F32, tag="mm", name="ps_k")
                    for kt in range(4):
                        nc.tensor.matmul(ps,
                                         wk[:, 2 * kt:2 * kt + 2,
                                            co * P:(co + 1) * P],
                                         raw[:, 2 * kt:2 * kt + 2, sl],
                                         start=(kt == 0), stop=(kt == 3),
                                         perf_mode=DR)
                    # two-op evac: psum-side multiply on DVE (gpsimd
                    # cannot touch PSUM), SBUF-side pos add on the idle
                    # Pool engine.
                    nc.vector.tensor_mul(kp[:, co, sl], ps, bc[:, sl])
                    nc.gpsimd.tensor_scalar_add(kp[:, co, sl], kp[:, co, sl],
                                                kpos_pp[:, co:co + 1])
            wv = load_w(f"wv_{m}")
            vp = vps.tile([P, KO, E], BF16, tag="vp", name=f"vp_{m}")
            for sch in range(KO):
                for dh in range(2):
                    sl = slice(dh * 512, (dh + 1) * 512)
                    ps = psA.tile([P, 512], F32, tag="mm", name="ps_v")
                    for kt in range(4):
                        nc.tensor.matmul(ps,
                                         raw[:, 2 * kt:2 * kt + 2,
                                             sch * P:(sch + 1) * P],
                                         wv[:, 2 * kt:2 * kt + 2, sl],
                                         start=(kt == 0), stop=(kt == 3),
                                         perf_mode=DR)
                    nc.vector.tensor_scalar(vp[:, sch, sl], ps,
                                            bcnT[:, sch:sch + 1], 1.0 / 16.0,
                                            mybir.AluOpType.mult,
                                            mybir.AluOpType.mult)
            return kp, vp

        def src_rhs_bf(src, ci, h4):
            if len(src.shape) == 4:
                return src[:, ci, h4 * 4:(h4 + 1) * 4, :]
            return src[:, ci, h4 * 512:(h4 + 1) * 512]

        def src_lhsT_bf(src, ci, s):
            if len(src.shape) == 4:
                return src[:, ci, s, :]
            return src[:, ci, s * P:(s + 1) * P]

        def kv_project(m, src, src_scale=1.0, src_v=None, v_scale=None,
                       v_first=False):
            """K/V projections over all 1024 rows.  fp8 sources run
            DoubleRow (kp comes out x16 fp8); bf16 sources run standard
            matmuls (kp bf16).  vp always at true scale (bf16)."""
            f8 = src.dtype == F8
            gain = src_scale * (16.0 if f8 else 1.0)
            if src_v is None:
                src_v = src
            v8 = src_v.dtype == F8
            vgain = v_scale if v_scale is not None else gain

            def do_k():
                wk = load_w(f"wk_{m}")
                kp = kps.tile([P, KO, E], F8 if f8 else BF16, tag="kp",
                              name=f"kp_{m}", padded_shape=[P, KO, E])
                for co in range(KO):
                    for h4 in range(2):
                        ps = psA.tile([P, 512], F32, tag="mm", name="ps_k")
                        if f8:
                            for kt in range(4):
                                nc.tensor.matmul(ps,
                                                 wk[:, 2 * kt:2 * kt + 2,
                                                    co * P:(co + 1) * P],
                                                 src_rhs(src, kt, h4),
                                                 start=(kt == 0),
                                                 stop=(kt == 3),
                                                 perf_mode=DR)
                        else:
                            for ci in range(KO):
                                nc.tensor.matmul(ps,
                                                 wk[:, ci,
                                                    co * P:(co + 1) * P],
                                                 src_rhs_bf(src, ci, h4),
                                                 start=(ci == 0),
                                                 stop=(ci == KO - 1))
                        sc = (16.0 if f8 else 1.0) / gain
                        nc.scalar.activation(
                            kp[:, co, h4 * 512:(h4 + 1) * 512], ps, AF.Copy,
                            scale=sc)
                return kp

            def do_v():
                wv = load_w(f"wv_{m}")
                vp = vps.tile([P, KO, E], BF16, tag="vp", name=f"vp_{m}")
                for s_ in range(KO):
                    for dh in range(2):
                        ps = psA.tile([P, 512], F32, tag="mm", name="ps_v")
                        if v8:
                            for kt in range(4):
                                nc.tensor.matmul(ps, src_lhsT(src_v, kt, s_),
                                                 wv[:, 2 * kt:2 * kt + 2,
                                                    dh * 512:(dh + 1) * 512],
                                                 start=(kt == 0),
                                                 stop=(kt == 3),
                                                 perf_mode=DR)
                        else:
                            for ci in range(KO):
                                nc.tensor.matmul(ps,
                                                 src_lhsT_bf(src_v, ci, s_),
                                                 wv[:, ci,
                                                    dh * 512:(dh + 1) * 512],
                                                 start=(ci == 0),
                                                 stop=(ci == KO - 1))
                        if vgain == 1.0:
                            nc.vector.tensor_copy(
                                vp[:, s_, dh * 512:(dh + 1) * 512], ps)
                        else:
                            nc.vector.tensor_scalar_mul(
                                vp[:, s_, dh * 512:(dh + 1) * 512], ps,
                                1.0 / vgain)
                return vp

            if v_first:
                vp = do_v()
                kp = do_k()
            else:
                kp = do_k()
                vp = do_v()
            return kp, vp

        def attention(m, qp, kp, vp):
            """-> ctxT [128, 8(ci), 128(L)] bf16 (pre-out-proj context)."""
            f8 = kp.dtype == F8
            expt = exps.tile([P, KO, 512], BF16, tag="exp", name=f"expt_{m}")
            for s in range(KO):
                ps = psA.tile([P, 512], F32, tag="mm", name="ps_sc")
                for h in range(4):
                    if f8:
                        nc.tensor.matmul(
                            ps[:, h * P:(h + 1) * P],
                            kp[:, 2 * h:2 * h + 2, s * P:(s + 1) * P],
                            qp[:, 2 * h:2 * h + 2], start=True, stop=True,
                            perf_mode=DR)
                    else:
                        for dk in range(2):
                            nc.tensor.matmul(
                                ps[:, h * P:(h + 1) * P],
                                kp[:, 2 * h + dk, s * P:(s + 1) * P],
                                qp[:, 2 * h + dk], start=(dk == 0),
                                stop=(dk == 1))
                # fp8 kp and qp both carry x16 -> scores x256
                nc.scalar.activation(expt[:, s], ps, AF.Exp,
                                     scale=0.0625 / 256.0 if f8 else 0.0625)
            pss = pssum.tile([1, 512], F32, tag="cs", name="ps_sm")
            for s in range(KO):
                nc.tensor.matmul(pss, ones_cb, expt[:, s], start=(s == 0),
                                 stop=(s == KO - 1))
            inv = smalls.tile([1, 512], F32R, tag="inv512", name="inv_sm")
            with nc.allow_low_precision(reason="softmax reciprocal"):
                nc.vector.reciprocal(inv, pss)
            bc = bcast_row(inv, 512, BF16, tag="bcs", name=f"bcs_{m}")
            for s in range(KO):
                nc.vector.tensor_mul(expt[:, s], expt[:, s], bc)
            ctx = ctxs.tile([P, E], BF16, tag="ctx", name=f"ctx_{m}")
            for hh in range(2):
                ps = psA.tile([P, 512], F32, tag="mm", name="ps_av")
                for hi in range(2):
                    h = 2 * hh + hi
                    for s in range(KO):
                        nc.tensor.matmul(
                            ps[:, hi * 256:(hi + 1) * 256],
                            expt[:, s, h * P:(h + 1) * P],
                            vp[:, s, h * 256:(h + 1) * 256],
                            start=(s == 0), stop=(s == KO - 1))
                nc.scalar.activation(ctx[:, hh * 512:(hh + 1) * 512], ps,
                                     AF.Copy)
            if DEBUG and m == "tg":
                nc.sync.dma_start(dbg["d_ctx_tg"], ctx)
            ctxT = ctxs.tile([P, KO, L], BF16, tag="ctxT", name=f"ctxT_{m}")
            for ci in range(KO):
                pt = psT.tile([P, P], BF16, tag="tr", name="ps_tr")
                nc.tensor.transpose(pt, ctx[:, ci * P:(ci + 1) * P], ident)
                nc.vector.tensor_copy(ctxT[:, ci], pt)
            return ctxT

        def out_proj(m, ctxT, residual, master=False, out_pool=None,
                     out_dt=BF16, oscale=1.0, bias=None):
            wo = load_w(f"wo_{m}")
            return gemm_own(wo, ctxT, bias or bias_pp[f"bo_{m}"], f"o_{m}",
                            residual=residual, master=master,
                            out_pool=out_pool, out_dt=out_dt, oscale=oscale)

        def dump_feat(nm, t):
            if DEBUG:
                nc.sync.dma_start(
                    dbg[nm].rearrange("(ko p) r -> p ko r", p=P), t)

        def dump_plain(nm, t):
            if DEBUG:
                nc.sync.dma_start(dbg[nm], t)

        def pack_piece(inbuf, sb_tile):
            # NB: collective buffers must be bf16/f32 -- f32r payloads get
            # mantissa-squashed by the collective transport in this runtime.
            nc.sync.dma_start(
                inbuf.rearrange("(p a b) -> p a b", p=P, a=KO), sb_tile)

        def allgather(inbuf, outbuf):
            nc.gpsimd.collective_compute(
                "AllGather", mybir.AluOpType.bypass,
                replica_groups=GROUPS8,
                ins=[inbuf.opt()], outs=[outbuf.opt()])

        def unpack_gather(outbuf, name):
            # ci-major gathered layout [128, 8(ci), 8(rank), 128] so that
            # DoubleRow k-tile slices are contiguous in dim 1.
            t = gath.tile([P, KO, KO, L], outbuf.dtype, tag="gf", name=name)
            for r in range(KO):
                nc.sync.dma_start(
                    t[:, :, r],
                    outbuf[r].rearrange("(p a b) -> p a b", p=P, a=KO))
            return t

        # ---------- stage 0 ----------
        # critical input DMAs go first; the 21 small bias loads would
        # otherwise serialize ~12us of SP-queue time ahead of them.
        raw_to = load_raw("x_text_own", L, big=False)
        raw_g = load_raw("x_glob", E, big=True)
        w_tg0 = load_w("w_tg")
        load_biases()
        textn_own = normalize("x_text_own", L, acts, tag="nto", raw=raw_to)
        inv_g = norm_inv(raw_g, E)
        bc_g = bcast_row(inv_g, E, BF16, tag="bcn1024", name="bc_g")
        bcnT_g = transpose_inv(inv_g, "bcnT_g")

        # ---------- tg path ----------
        w_tg = w_tg0
        t_g_f8, t_g32 = gemm_own(w_tg, textn_own, bias_pp["b_tg"], "t_g",
                                 master=True, out_pool=pers2, out_dt=F8,
                                 oscale=32.0)
        wq_tg = load_w("wq_tg")
        qp_tg, _ = gemm_own(wq_tg, t_g_f8, bias_pp["bq_tg"], "qp_tg",
                            out_pool=qps, out_dt=F8, oscale=16.0,
                            in_scale=512.0, perf_mode=DR)
        kp_tg, vp_tg = kv_project_raw("tg", raw_g, bc_g, bcnT_g,
                                      bias_pp["kpos_tg"])
        ctxT_tg = attention("tg", qp_tg, kp_tg, vp_tg)
        gt_bf, _ = out_proj("tg", ctxT_tg, t_g32, master=True, out_dt=F8,
                            oscale=32.0, bias=bias_pp["bp_tg"])
        dump_feat("d_textn_own", textn_own)
        dump_feat("d_t_g", t_g_f8)
        dump_feat("d_qp_tg", qp_tg)
        dump_feat("d_kp_tg", kp_tg)
        dump_feat("d_vp_tg", vp_tg)
        dump_feat("d_gt", gt_bf)

        in1 = dram_p.tile([PIECE], F8, name="in1")
        out1 = dram_p.tile([KO, PIECE], F8, name="out1")
        pack_piece(in1, gt_bf)
        allgather(in1, out1)

        # ---------- tl path (overlaps gt gather) ----------
        raw_l = load_raw("x_loc", E, big=True)
        inv_l = norm_inv(raw_l, E)
        bc_l = bcast_row(inv_l, E, BF16, tag="bcn1024", name="bc_l")
        bcnT_l = transpose_inv(inv_l, "bcnT_l")
        w_tl = load_w("w_tl")
        t_l_f8, t_l32 = gemm_own(w_tl, textn_own, bias_pp["b_tl"], "t_l",
                                 master=True, out_pool=pers2, out_dt=F8,
                                 oscale=32.0)
        wq_tl = load_w("wq_tl")
        qp_tl, _ = gemm_own(wq_tl, t_l_f8, bias_pp["bq_tl"], "qp_tl",
                            out_pool=qps, out_dt=F8, oscale=16.0,
                            in_scale=512.0, perf_mode=DR)
        kp_tl, vp_tl = kv_project_raw("tl", raw_l, bc_l, bcnT_l,
                                      bias_pp["kpos_tl"])
        ctxT_tl = attention("tl", qp_tl, kp_tl, vp_tl)
        lt_f8, lt32 = out_proj("tl", ctxT_tl, t_l32, master=True,
                               out_pool=pers2, out_dt=F8, oscale=32.0)
        wq_ff = load_w("wq_ff")
        qp_ff, _ = gemm_own(wq_ff, lt_f8, bias_pp["bq_ff"], "qp_ff",
                            out_pool=qps, out_dt=F8, oscale=16.0,
                            in_scale=512.0, perf_mode=DR)
        # full-text norm scale (for t_r in the next window); the normalized
        # text itself is never materialized -- the scale commutes through
        # the t_r GEMM.
        raw_text = load_raw("x_text", E, big=True)
        inv_text = norm_inv(raw_text, E)
        bc_text = bcast_row(inv_text, E, F32R, tag="bct", name="bc_text")

        # ---------- ff MHA ----------
        gt_full = unpack_gather(out1, "gt_full")
        if DEBUG:
            for r in range(KO):
                nc.sync.dma_start(
                    dbg["d_gtf"][r].rearrange("(ko p) l -> p ko l", p=P),
                    gt_full[:, :, r])
        kp_ff, vp_ff = kv_project("ff", gt_full, 32.0)
        ctxT_ff = attention("ff", qp_ff, kp_ff, vp_ff)
        ff_bf, _ = out_proj("ff", ctxT_ff, lt32, bias=bias_pp["bp_ff"])
        dump_feat("d_lt", lt32)
        dump_feat("d_ff", ff_bf)

        in2 = dram_p.tile([PIECE], BF16, name="in2")
        out2 = dram_p.tile([KO, PIECE], BF16, name="out2")
        pack_piece(in2, ff_bf)
        allgather(in2, out2)

        # ---------- window 2 (overlaps ff gather): t_r + final prep ----------
        w_rep = load_w("w_rep")
        t_r = pers.tile([P, KO, E], BF16, name="t_r")
        for co in range(KO):
            for h4 in range(2):
                ps = psA.tile([P, 512], F32, tag="mm", name="ps_tr2")
                for ci in range(KO):
                    nc.tensor.matmul(ps, w_rep[:, ci, co * P:(co + 1) * P],
                                     raw_text[:, ci, h4 * 512:(h4 + 1) * 512],
                                     start=(ci == 0), stop=(ci == KO - 1))
                sl = t_r[:, co, h4 * 512:(h4 + 1) * 512]
                nc.vector.tensor_mul(sl, ps, bc_text[:, h4 * 512:(h4 + 1) * 512])
                nc.vector.tensor_scalar_add(sl, sl,
                                            bias_pp["b_rep"][:, co:co + 1])
        t_r_own, _ = gemm_own(w_rep, textn_own, bias_pp["b_rep"], "t_r_own",
                              out_dt=F8, oscale=32.0)
        wq_rt = load_w("wq_rt")
        qp_rt, _ = gemm_own(wq_rt, t_r_own, bias_pp["bq_rt"], "qp_rt",
                            out_pool=qps, out_dt=F8, oscale=16.0,
                            in_scale=512.0, perf_mode=DR)
        lfn = normalize("x_loc_grp", GRP, pers, tag="lfn", big=True)

        # ---------- rt MHA ----------
        ff_full = unpack_gather(out2, "ff_full")
        # fp8 x32 copy for the K/Q path (softmax washes fp8 noise); the
        # V path keeps the bf16 original.  Converted per rank chunk so it
        # pipelines with the unpack DMAs; V projection is emitted first so
        # the PE starts on bf16 V work while the conversion drains.
        ff_f8 = gath.tile([P, KO, KO, L], F8, tag="gf8", name="ff_f8")
        for r in range(KO):
            nc.gpsimd.tensor_scalar_mul(ff_f8[:, :, r], ff_full[:, :, r],
                                        32.0)
        kp_rt, vp_rt = kv_project("rt", ff_f8, 32.0, src_v=ff_full,
                                  v_scale=1.0, v_first=True)
        ctxT_rt = attention("rt", qp_rt, kp_rt, vp_rt)
        rt_bf, _ = out_proj("rt", ctxT_rt, None)
        dump_feat("d_t_r", t_r)
        dump_feat("d_rt", rt_bf)
        dump_feat("d_lfn", lfn)

        # ---------- final: full = rt @ t_r.T, cosine logits ----------
        # row-major full (for row norms): out[q(part), c] = sum_e rt[e,q] t_r[e,c]
        sq_scratch = finals.tile([P, 512], BF16, tag="fsq", name="fsq")
        frow = finals.tile([P, E], BF16, tag="frow", name="frow")
        acc = finals.tile([P, 2], F32, tag="acc2", name="acc_rn")
        for h4 in range(2):
            ps = psA.tile([P, 512], F32, tag="mm", name="ps_fr")
            for ci in range(KO):
                nc.tensor.matmul(ps, rt_bf[:, ci],
                                 t_r[:, ci, h4 * 512:(h4 + 1) * 512],
                                 start=(ci == 0), stop=(ci == KO - 1))
            nc.scalar.activation(frow[:, h4 * 512:(h4 + 1) * 512], ps, AF.Copy)
            nc.scalar.activation(sq_scratch, ps, AF.Square,
                                 accum_out=acc[:, h4:h4 + 1])
        rn = finals.tile([P, 1], F32, tag="rn", name="rn")
        nc.vector.tensor_add(rn, acc[:, 0:1], acc[:, 1:2])
        nc.scalar.sqrt(rn, rn)
        nc.vector.tensor_scalar_max(rn, rn, EPS)
        inv_q = finals.tile([P, 1], F32, tag="invq", name="inv_q")
        nc.vector.reciprocal(inv_q, rn)

        # feat-major fullT (logits lhsT) via PE transpose of full_row
        fullT = finals.tile([P, KO, L], BF16, tag="fullT", name="fullT")
        for cc in range(KO):
            pt = psT.tile([P, P], BF16, tag="tr", name="ps_ftr")
            nc.tensor.transpose(pt, frow[:, cc * P:(cc + 1) * P], ident)
            nc.vector.tensor_copy(fullT[:, cc], pt)

        dump_plain("d_frow", frow)
        lg = finals.tile([P, GRP], F32, tag="lg", name="lg")
        ps = psA.tile([P, 512], F32, tag="mm", name="ps_lg")
        for cc in range(KO):
            nc.tensor.matmul(ps[:, :GRP], fullT[:, cc], lfn[:, cc],
                             start=(cc == 0), stop=(cc == KO - 1))
        nc.vector.tensor_scalar_mul(lg, ps[:, :GRP], inv_q)
        nc.sync.dma_start(out_logits, lg)

    nc.compile()
    return nc


def make_in_maps(local_feat, global_feat, text_feat,
                 w_tl, b_tl, w_tg, b_tg, w_rep, b_rep,
                 pos_local, pos_global, mha_params):
    f32 = np.float32
    bf16 = ml_dtypes.bfloat16
    f8 = ml_dtypes.float8_e4m3
    textT = np.ascontiguousarray(text_feat.T.astype(bf16))
    locT = np.ascontiguousarray(local_feat.T.astype(bf16))
    shared = {
        "x_text": textT,
        "x_loc": np.ascontiguousarray(local_feat.T.astype(f8)),
        "x_glob": np.ascontiguousarray(global_feat.T.astype(f8)),
        "w_tl": np.ascontiguousarray(w_tl.T.astype(bf16)),
        "w_tg": np.ascontiguousarray(w_tg.T.astype(bf16)),
        "w_rep": np.ascontiguousarray(w_rep.T.astype(bf16)),
        "b_tl": b_tl.astype(f32), "b_tg": b_tg.astype(f32),
        "b_rep": b_rep.astype(f32),
    }
    wv_f, wo_f, bo_eff = {}, {}, {}
    for m, (wi, bi, wo, bo) in mha_params.items():
        # q/k/v weights in fp8 (x16 into the e4m3 sweet spot), except the
        # precision-critical rt block which stays bf16
        shared[f"wq_{m}"] = np.ascontiguousarray(
            (16.0 * wi[0 * E:1 * E].T).astype(f8))
        shared[f"wk_{m}"] = np.ascontiguousarray(
            (16.0 * wi[1 * E:2 * E].T).astype(f8))
        if m == "rt":
            shared[f"wv_{m}"] = np.ascontiguousarray(
                wi[2 * E:3 * E].T.astype(bf16))
        else:
            shared[f"wv_{m}"] = np.ascontiguousarray(
                (16.0 * wi[2 * E:3 * E].T).astype(f8))
        shared[f"wo_{m}"] = np.ascontiguousarray(wo.T.astype(bf16))
        shared[f"bq_{m}"] = bi[0 * E:1 * E].astype(f32)
        wv_f[m], wo_f[m] = wi[2 * E:3 * E], wo
        # V bias folded into output-projection bias: bo_eff = bo + wo @ bv
        bo_eff[m] = bo + wo @ bi[2 * E:3 * E]
    # The V projections run on pos-free / mean-shifted sources; each removed
    # constant c contributes wo @ (wv @ c) to the block's output bias:
    #   tl/tg: V source excludes pos_local/pos_global
    #   ff:    consumes gt' = gt - c_g   (c_g ~ mean over rows of gt)
    #   rt:    consumes ff' = ff - c_f
    bo_eff["tl"] = bo_eff["tl"] + wo_f["tl"] @ (wv_f["tl"] @ pos_local)
    bo_eff["tg"] = bo_eff["tg"] + wo_f["tg"] @ (wv_f["tg"] @ pos_global)
    c_g = bo_eff["tg"] + b_tg
    bo_eff["ff"] = bo_eff["ff"] + wo_f["ff"] @ (wv_f["ff"] @ c_g)
    c_f = bo_eff["ff"] + bo_eff["tl"] + b_tl
    bo_eff["rt"] = bo_eff["rt"] + wo_f["rt"] @ (wv_f["rt"] @ c_f)
    for m in mha_params:
        shared[f"bo_{m}"] = bo_eff[m].astype(f32)
    shared["bp_tg"] = (bo_eff["tg"] - c_g).astype(f32)
    shared["bp_ff"] = (bo_eff["ff"] - c_f).astype(f32)
    # host-folded K-projection pos terms (x16 to match fp8 kp scaling)
    shared["kpos_tl"] = (16.0 * (mha_params["tl"][0][E:2 * E] @
                                 pos_local)).astype(f32)
    shared["kpos_tg"] = (16.0 * (mha_params["tg"][0][E:2 * E] @
                                 pos_global)).astype(f32)

    in_maps = []
    for c in range(NCORES):
        g = c // 2
        m = dict(shared)
        m["x_text_own"] = np.ascontiguousarray(textT[:, c * L:(c + 1) * L])
        m["x_loc_grp"] = np.ascontiguousarray(locT[:, g * GRP:(g + 1) * GRP])
        in_maps.append(m)
    return in_maps


def kernel(local_feat, global_feat, text_feat,
           w_tl, b_tl, w_tg, b_tg, w_rep, b_rep,
           pos_local, pos_global,
           tl_wi, tl_bi, tl_wo, tl_bo,
           tg_wi, tg_bi, tg_wo, tg_bo,
           ff_wi, ff_bi, ff_wo, ff_bo,
           rt_wi, rt_bi, rt_wo, rt_bo,
           n_groups):
    assert int(n_groups) == 4
    if "nc" not in _CACHE:
        _CACHE["nc"] = build_nc()
    nc = _CACHE["nc"]
    mha_params = {
        "tl": (tl_wi, tl_bi, tl_wo, tl_bo),
        "tg": (tg_wi, tg_bi, tg_wo, tg_bo),
        "ff": (ff_wi, ff_bi, ff_wo, ff_bo),
        "rt": (rt_wi, rt_bi, rt_wo, rt_bo),
    }
    in_maps = make_in_maps(np.asarray(local_feat), np.asarray(global_feat),
                           np.asarray(text_feat),
                           np.asarray(w_tl), np.asarray(b_tl),
                           np.asarray(w_tg), np.asarray(b_tg),
                           np.asarray(w_rep), np.asarray(b_rep),
                           np.asarray(pos_local), np.asarray(pos_global),
                           {k: tuple(np.asarray(x) for x in v)
                            for k, v in mha_params.items()})
    res = run_bass_kernel_spmd(nc, in_maps, core_ids=list(range(NCORES)))
    _CACHE["last_results"] = res
    out = np.empty((4, GRP, GRP), dtype=np.float32)
    for c in range(NCORES):
        g, half = c // 2, c % 2
        out[g, half * L:(half + 1) * L, :] = res.results[c]["logits"]
    return out


# revision 67
# speedup vs baseline: 2.6952x; 1.0115x over previous
"""Trainium2 Bass kernel for nn_Model4 (retrieval_knn).

Model: 3 l2-normalized feature streams -> 4 chained MultiheadAttention blocks
-> full = rt @ t_r.T -> per-group cosine logits [4, 256, 256].

Sharding (v2): 8-way row sharding (core c owns rows [128c, 128c+128)) with
REPLICATED K/V projections.  K/V sources for the tl/tg MHAs (local_n+pos,
global_n+pos) derive from inputs, so every core computes full-sequence K/V
locally; only the two intermediate activations that cross MHAs (gt, ff) are
AllGather'd.  That cuts the collective count from 6 (614us) to 2 (135us), and
both gathers overlap with independent compute (tl-MHA during the gt gather,
t_r/final-stage prep during the ff gather).

Bias algebra: K-projection bias drops out of softmax (adds a per-query
constant to every score); V-projection bias is folded into the output
projection bias host-side (bo_eff = bo + wo @ bv).

Layouts: activations feat-major ([chan(part) x chunks, rows(free)]); V
projections row-major ([S(part) x chunks, dv(free)]) so they serve as AV
lhsT directly; attention uses transposed-softmax (no max subtraction; scores
are ~1e-3).  Gathered tensors keep rank-major S order == global row order.
The full-text normalization scale commutes through the t_r GEMM (columns
scaled post-GEMM), so normalized full text is never materialized.
"""
import sys

sys.path.insert(0, "/opt/trn_rl_repo")

import ml_dtypes
import numpy as np

import concourse.bass as bass  # noqa: F401
import concourse.tile as tile
import concourse.mybir as mybir
from concourse import bacc
from concourse.bass_utils import run_bass_kernel_spmd
from concourse.masks import make_identity

E = 1024
P = 128
KO = 8               # feature chunks of 128
L = 128              # rows per core
GRP = 256            # rows per output group
NCORES = 8
F32 = mybir.dt.float32
F32R = mybir.dt.float32r
BF16 = mybir.dt.bfloat16
F8 = mybir.dt.float8e4
DR = mybir.MatmulPerfMode.DoubleRow
AF = mybir.ActivationFunctionType
GROUPS8 = [[0, 1, 2, 3, 4, 5, 6, 7]]
EPS = 1e-8
PIECE = P * KO * L   # 131072 bf16 elements in one packed [128,8,128] piece

DEBUG = False
_CACHE = {}


def build_nc():
    nc = bacc.Bacc("TRN2", target_bir_lowering=False, debug=False,
                   num_devices=NCORES)
    dram = {}

    def din(name, shape, dt=BF16):
        dram[name] = nc.dram_tensor(name, shape, dt, kind="ExternalInput").ap()

    # full feat-major feature streams; glob/loc only feed the K/V
    # projections so they ship as raw fp8 (values are ~N(0,1))
    din("x_glob", [E, E], F8)
    din("x_text", [E, E])
    din("x_loc", [E, E], F8)
    # per-core slices
    din("x_text_own", [E, L])
    din("x_loc_grp", [E, GRP])
    # shared projections, host-transposed to [cin, cout]
    for w in ("w_tl", "w_tg", "w_rep"):
        din(w, [E, E])
    for b in ("b_tl", "b_tg", "b_rep"):
        din(b, [E], F32)
    # host-folded K-projection pos terms: 16 * (wk @ pos)
    din("kpos_tl", [E], F32)
    din("kpos_tg", [E], F32)
    # per-MHA weights, host-transposed to [cin, cout]; K bias dropped,
    # V bias folded into bo host-side.  Q/K/V weights in fp8 (x16 host
    # scaling), output projection in bf16.
    for m in ("tl", "tg", "ff", "rt"):
        for w in ("wq", "wk", "wv"):
            # rt's V path stays bf16: its output has no residual protection,
            # so fp8 V-side noise there would land directly on the logits.
            # rt's K/Q path is fp8 (softmax washes that noise out).
            din(f"{w}_{m}", [E, E], BF16 if (m, w) == ("rt", "wv") else F8)
        din(f"wo_{m}", [E, E])
        din(f"bq_{m}", [E], F32)
        din(f"bo_{m}", [E], F32)

    din("bp_tg", [E], F32)    # packed-gt bias:  bo_eff_tg - c_g
    din("bp_ff", [E], F32)    # packed-ff bias:  bo_eff_ff - c_f
    out_logits = nc.dram_tensor("logits", [L, GRP], F32,
                                kind="ExternalOutput").ap()
    dbg = {}
    if DEBUG:
        for nm, shape, dt in [
                ("d_kvg", [E, E], F8), ("d_textn_own", [E, L], BF16),
                ("d_t_g", [E, L], F8), ("d_qp_tg", [E, L], F8),
                ("d_kp_tg", [E, E], F8), ("d_vp_tg", [E, E], BF16),
                ("d_ctx_tg", [L, E], BF16), ("d_gt", [E, L], F8),
                ("d_gtf", [KO, E, L], F8), ("d_lt", [E, L], F32),
                ("d_ff", [E, L], BF16), ("d_rt", [E, L], BF16),
                ("d_t_r", [E, E], BF16), ("d_frow", [L, E], BF16),
                ("d_lfn", [E, GRP], BF16)]:
            dbg[nm] = nc.dram_tensor(nm, shape, dt,
                                     kind="ExternalOutput").ap()

    from contextlib import ExitStack
    with tile.TileContext(nc) as tc, ExitStack() as ctx:
        def pool(name, bufs, space="SBUF"):
            return ctx.enter_context(
                tc.tile_pool(name=name, bufs=bufs, space=space))

        consts = pool("consts", 1)
        raws = pool("raws", 1)       # one big raw stream buffer (16k)
        rawsm = pool("rawsm", 1)     # small raw slices (3k)
        acts = pool("acts", 1)       # textn_own (2k)
        gath = pool("gath", 1)       # gathered gt/ff (16k + 8k fp8 copy)
        kps = pool("kps", 1)         # K proj (16k)
        vps = pool("vps", 1)         # V proj (16k)
        exps = pool("exps", 1)       # exp scores (8k)
        ctxs = pool("ctxs", 1)       # ctx + ctxT (4k)
        pers = pool("pers", 1)       # t_r (16k) + lfn (4k)
        bcs = pool("bcs", 1)         # broadcast tiles (~10k)
        smalls = pool("smalls", 1)   # inv/nrm rows (~10k)
        finals = pool("finals", 1)   # final-stage tiles (~5k)
        sqs = pool("sqs", 2)         # squared chunks (2k x2)
        pers2 = pool("pers2", 2)     # f32 masters t_g/t_l/lt (12k)
        qps = pool("qps", 2)         # q projections (4k)
        outs_p = pool("outs_p", 2)   # MHA outputs pre-pack (4k)
        weights = pool("weights", 3)  # streamed weights (48k)
        psA = pool("psA", 5, space="PSUM")
        pssum = pool("pssum", 1, space="PSUM")
        psT = pool("psT", 2, space="PSUM")
        dram_p = pool("dram_p", 1, space="DRAM")

        # ---------- constants ----------
        ones_cb = consts.tile([P, 1], BF16)
        nc.vector.memset(ones_cb, 1.0)
        ones_cf32 = consts.tile([P, 1], F32)
        nc.vector.memset(ones_cf32, 1.0)
        ones_cr = consts.tile([P, 1], F32R)
        nc.vector.tensor_copy(ones_cr, ones_cf32)
        ones_rf32 = consts.tile([1, P], F32)
        nc.vector.memset(ones_rf32, 1.0)
        ones_rr = consts.tile([1, P], F32R)
        nc.vector.tensor_copy(ones_rr, ones_rf32)
        ident = consts.tile([P, P], BF16)
        make_identity(nc, ident)

        def load_bias_pp(name):
            t = consts.tile([P, KO], F32, name=f"c_{name}")
            nc.sync.dma_start(t, dram[name].rearrange("(c p) -> p c", p=P))
            return t

        bias_pp = {}

        def load_biases():
            for nm in ("b_tl", "b_tg", "b_rep", "kpos_tl", "kpos_tg"):
                bias_pp[nm] = load_bias_pp(nm)
            for m in ("tl", "tg", "ff", "rt"):
                bias_pp[f"bq_{m}"] = load_bias_pp(f"bq_{m}")
                bias_pp[f"bo_{m}"] = load_bias_pp(f"bo_{m}")
            bias_pp["bp_tg"] = load_bias_pp("bp_tg")
            bias_pp["bp_ff"] = load_bias_pp("bp_ff")

        # ---------- helpers ----------
        def load_w(name):
            """[1024, 1024] dram -> [128, 8, 1024] (p, cin-chunk, cout)."""
            dt = dram[name].dtype
            t = weights.tile([P, KO, E], dt, tag="w", name=f"w_{name}",
                             padded_shape=[P, KO, E] if dt == F8 else None)
            nc.sync.dma_start(t, dram[name].rearrange("(ko p) c -> p ko c",
                                                      p=P))
            return t

        def norm_inv(raw, n, ncols=None, scale=1.0):
            """Per-free-column scale/l2norm over all 8 chunks of raw
            [128,8,n].  Returns [1, n] f32r."""
            ncols = ncols or n
            nhalf = (ncols + 511) // 512
            nrm = smalls.tile([1, ncols], F32R, tag="nrm", name="nrm",
                              padded_shape=[1, E])
            inv = nrm
            for h in range(nhalf):
                lo = h * 512
                hi = min(ncols, lo + 512)
                pss = pssum.tile([1, min(512, ncols)], F32, tag="cs",
                                 name="ps_cs")
                for ko in range(KO):
                    sq = sqs.tile([P, 512], F32R, tag="sq", name="sq",
                                  padded_shape=[P, 512])
                    nc.scalar.activation(sq[:, :hi - lo], raw[:, ko, lo:hi],
                                         AF.Square)
                    nc.tensor.matmul(pss[:, :hi - lo], ones_cr,
                                     sq[:, :hi - lo], start=(ko == 0),
                                     stop=(ko == KO - 1))
                # sqrt(x / scale^2) = ||x|| / scale  ->  inv = scale/||x||
                nc.scalar.activation(nrm[:, lo:hi], pss[:, :hi - lo],
                                     AF.Sqrt, scale=1.0 / (scale * scale))
            with nc.allow_low_precision(reason="norm reciprocal"):
                nc.vector.reciprocal(inv, nrm)
            return inv


        def bcast_row(row_r, n, dtype, tag, name="bc"):
            """[1, n] f32r -> [128, n] broadcast tile."""
            out = bcs.tile([P, n], dtype, tag=tag, name=name)
            for h in range((n + 511) // 512):
                lo = h * 512
                hi = min(n, lo + 512)
                ps = psA.tile([P, 512], F32, tag="mm", name="ps_bc")
                nc.tensor.matmul(ps[:, :hi - lo], ones_rr, row_r[:, lo:hi],
                                 start=True, stop=True)
                nc.scalar.activation(out[:, lo:hi], ps[:, :hi - lo], AF.Copy)
            return out

        def load_raw(xname, n, big):
            p = raws if big else rawsm
            raw = p.tile([P, KO, n], dram[xname].dtype,
                         tag="raw1024" if big else f"raw{n}",
                         name=f"raw_{xname}",
                         padded_shape=[P, KO, E] if big else None)
            for ko in range(KO):
                nc.sync.dma_start(raw[:, ko],
                                  dram[xname][ko * P:(ko + 1) * P, :])
            return raw

        def normalize(xname, n, out_pool, pos=None, tag=None, big=False,
                      dtype=BF16, scale=1.0, raw=None):
            """bf16 feat-major [E, n] dram -> scale*l2norm rows, [128,8,n].
            With pos: returns (k_tile with pos added, v_tile without pos) --
            the V path excludes the constant pos component (folded into the
            output-projection bias host-side) so that systematic fp8
            weight-quantization noise has no constant carrier."""
            if raw is None:
                raw = load_raw(xname, n, big)
            inv = norm_inv(raw, n, scale=scale)
            bc = bcast_row(inv, n, F32R, tag=f"bcn{n}", name=f"bc_{xname}")
            out = out_pool.tile([P, KO, n], dtype, tag=tag or f"nb{n}",
                                name=f"n_{xname}")
            for ko in range(KO):
                nc.vector.tensor_mul(out[:, ko], raw[:, ko], bc)
            if pos is None:
                return out
            outk = out_pool.tile([P, KO, n], dtype, tag=(tag or f"nb{n}") + "k",
                                 name=f"nk_{xname}")
            for ko in range(KO):
                nc.vector.tensor_scalar_add(outk[:, ko], out[:, ko],
                                            pos[:, ko:ko + 1])
            return outk, out

        def gemm_own(w_sb, src_bf, bias, name, residual=None, master=False,
                     out_pool=None, out_dt=BF16, oscale=1.0, in_scale=1.0,
                     perf_mode=None):
            """Own-rows GEMM via fat row-major matmuls (lhsT = feat-major
            src), then PE-transpose back to feat-major [128, 8(co), 128(L)].
            Emits out = ((psum/in_scale) + bias) * oscale (+ residual, which
            must already be in oscale).  With perf_mode=DR, src/w are fp8
            and contraction runs two 128-chunks per matmul."""
            pool_ = out_pool or outs_p
            out_bf = pool_.tile([P, KO, L], out_dt, tag="ob", name=name,
                                padded_shape=[P, KO, L])
            out_f = None
            if master:
                # master stays at TRUE scale; the secondary copy applies
                # oscale (e.g. x32 into fp8).
                out_f = pool_.tile([P, KO, L], F32, tag="of", name=name + "_f")
            row = ctxs.tile([P, E], BF16, tag="grow", name=f"row_{name}")
            pss = [psA.tile([P, 512], F32, tag="mm", name=f"ps_go{h}")
                   for h in range(2)]
            nk = 4 if perf_mode is DR else KO
            for ci in range(nk):
                for h in range(2):
                    if perf_mode is DR:
                        nc.tensor.matmul(pss[h], src_bf[:, 2 * ci:2 * ci + 2],
                                         w_sb[:, 2 * ci:2 * ci + 2,
                                              h * 512:(h + 1) * 512],
                                         start=(ci == 0), stop=(ci == nk - 1),
                                         perf_mode=DR)
                    else:
                        nc.tensor.matmul(pss[h], src_bf[:, ci],
                                         w_sb[:, ci, h * 512:(h + 1) * 512],
                                         start=(ci == 0), stop=(ci == nk - 1))
            for h in range(2):
                nc.scalar.activation(row[:, h * 512:(h + 1) * 512], pss[h],
                                     AF.Copy, scale=1.0 / in_scale)
            for co in range(KO):
                pt = psT.tile([P, P], BF16, tag="tr", name="ps_gt")
                nc.tensor.transpose(pt, row[:, co * P:(co + 1) * P], ident)
                tgt = out_f if master else out_bf
                mscale = 1.0 if master else oscale
                if bias is not None:
                    if mscale != 1.0:
                        nc.vector.tensor_scalar(tgt[:, co], pt,
                                                bias[:, co:co + 1], mscale,
                                                mybir.AluOpType.add,
                                                mybir.AluOpType.mult)
                    else:
                        nc.vector.tensor_scalar_add(tgt[:, co], pt,
                                                    bias[:, co:co + 1])
                    if residual is not None:
                        nc.vector.tensor_add(tgt[:, co], tgt[:, co],
                                             residual[:, co])
                elif residual is not None:
                    nc.vector.tensor_add(tgt[:, co], pt, residual[:, co])
                else:
                    nc.vector.tensor_copy(tgt[:, co], pt)
                if master:
                    nc.scalar.activation(out_bf[:, co], out_f[:, co], AF.Copy,
                                         scale=oscale)
            return out_bf, out_f

        # fp8 kv source accessors (k-tile pairs for DoubleRow):
        # plain [128, 8(ci), 1024(S)] or gathered ci-major
        # [128, 8(ci), 8(rank), 128]
        def src_rhs(src, kt, h4):
            if len(src.shape) == 4:
                return src[:, 2 * kt:2 * kt + 2, h4 * 4:(h4 + 1) * 4, :]
            return src[:, 2 * kt:2 * kt + 2, h4 * 512:(h4 + 1) * 512]

        def src_lhsT(src, kt, s):
            if len(src.shape) == 4:
                return src[:, 2 * kt:2 * kt + 2, s, :]
            return src[:, 2 * kt:2 * kt + 2, s * P:(s + 1) * P]

        def transpose_inv(inv, name):
            """[1, E] f32r row -> [128, 8] per-partition scalars, via a tiny
            DRAM round-trip (strided reload transposes for free)."""
            scratch = dram_p.tile([E], F32, name=name + "_d")
            nc.sync.dma_start(scratch, inv.bitcast(F32))
            t = smalls.tile([P, KO], F32, tag="bcnT", name=name)
            nc.sync.dma_start(t, scratch.rearrange("(a p) -> p a", p=P))
            return t

        def kv_project_raw(m, raw, bc, bcnT, kpos_pp):
            """K/V projections straight from the raw fp8 stream; the
            normalization scale is applied per-column at evacuation and the
            pos contribution to K is a host-folded per-chan constant.
            kp = 16*true (fp8); vp = true (bf16)."""
            wk = load_w(f"wk_{m}")
            kp = kps.tile([P, KO, E], F8, tag="kp", name=f"kp_{m}",
                          padded_shape=[P, KO, E])
            for co in range(KO):
                for h4 in range(2):
                    sl = slice(h4 * 512, (h4 + 1) * 512)
                    ps = psA.tile([P, 512], F32, tag="mm", name="ps_k")
                    for kt in range(4):
                        nc.tensor.matmul(ps,
                                         wk[:, 2 * kt:2 * kt + 2,
                                            co * P:(co + 1) * P],
                                         raw[:, 2 * kt:2 * kt + 2, sl],
                                         start=(kt == 0), stop=(kt == 3),
                                         perf_mode=DR)
                    # two-op evac: psum-side multiply on DVE (gpsimd
                    # cannot touch PSUM), SBUF-side pos add on the idle
                    # Pool engine.
                    nc.vector.tensor_mul(kp[:, co, sl], ps, bc[:, sl])
                    nc.gpsimd.tensor_scalar_add(kp[:, co, sl], kp[:, co, sl],
                                                kpos_pp[:, co:co + 1])
            wv = load_w(f"wv_{m}")
            vp = vps.tile([P, KO, E], BF16, tag="vp", name=f"vp_{m}")
            for sch in range(KO):
                for dh in range(2):
                    sl = slice(dh * 512, (dh + 1) * 512)
                    ps = psA.tile([P, 512], F32, tag="mm", name="ps_v")
                    for kt in range(4):
                        nc.tensor.matmul(ps,
                                         raw[:, 2 * kt:2 * kt + 2,
                                             sch * P:(sch + 1) * P],
                                         wv[:, 2 * kt:2 * kt + 2, sl],
                                         start=(kt == 0), stop=(kt == 3),
                                         perf_mode=DR)
                    nc.vector.tensor_scalar(vp[:, sch, sl], ps,
                                            bcnT[:, sch:sch + 1], 1.0 / 16.0,
                                            mybir.AluOpType.mult,
                                            mybir.AluOpType.mult)
            return kp, vp

        def src_rhs_bf(src, ci, h4):
            if len(src.shape) == 4:
                return src[:, ci, h4 * 4:(h4 + 1) * 4, :]
            return src[:, ci, h4 * 512:(h4 + 1) * 512]

        def src_lhsT_bf(src, ci, s):
            if len(src.shape) == 4:
                return src[:, ci, s, :]
            return src[:, ci, s * P:(s + 1) * P]

        def kv_project(m, src, src_scale=1.0, src_v=None, v_scale=None,
                       v_first=False):
            """K/V projections over all 1024 rows.  fp8 sources run
            DoubleRow (kp comes out x16 fp8); bf16 sources run standard
            matmuls (kp bf16).  vp always at true scale (bf16)."""
            f8 = src.dtype == F8
            gain = src_scale * (16.0 if f8 else 1.0)
            if src_v is None:
                src_v = src
            v8 = src_v.dtype == F8
            vgain = v_scale if v_scale is not None else gain

            def do_k():
                wk = load_w(f"wk_{m}")
                kp = kps.tile([P, KO, E], F8 if f8 else BF16, tag="kp",
                              name=f"kp_{m}", padded_shape=[P, KO, E])
                for co in range(KO):
                    for h4 in range(2):
                        ps = psA.tile([P, 512], F32, tag="mm", name="ps_k")
                        if f8:
                            for kt in range(4):
                                nc.tensor.matmul(ps,
                                                 wk[:, 2 * kt:2 * kt + 2,
                                                    co * P:(co + 1) * P],
                                                 src_rhs(src, kt, h4),
                                                 start=(kt == 0),
                                                 stop=(kt == 3),
                                                 perf_mode=DR)
                        else:
                            for ci in range(KO):
                                nc.tensor.matmul(ps,
                                                 wk[:, ci,
                                                    co * P:(co + 1) * P],
                                                 src_rhs_bf(src, ci, h4),
                                                 start=(ci == 0),
                                                 stop=(ci == KO - 1))
                        sc = (16.0 if f8 else 1.0) / gain
                        nc.scalar.activation(
                            kp[:, co, h4 * 512:(h4 + 1) * 512], ps, AF.Copy,
                            scale=sc)
                return kp

            def do_v():
                wv = load_w(f"wv_{m}")
                vp = vps.tile([P, KO, E], BF16, tag="vp", name=f"vp_{m}")
                for s_ in range(KO):
                    for dh in range(2):
                        ps = psA.tile([P, 512], F32, tag="mm", name="ps_v")
                        if v8:
                            for kt in range(4):
                                nc.tensor.matmul(ps, src_lhsT(src_v, kt, s_),
                                                 wv[:, 2 * kt:2 * kt + 2,
                                                    dh * 512:(dh + 1) * 512],
                                                 start=(kt == 0),
                                                 stop=(kt == 3),
                                                 perf_mode=DR)
                        else:
                            for ci in range(KO):
                                nc.tensor.matmul(ps,
                                                 src_lhsT_bf(src_v, ci, s_),
                                                 wv[:, ci,
                                                    dh * 512:(dh + 1) * 512],
                                                 start=(ci == 0),
                                                 stop=(ci == KO - 1))
                        if vgain == 1.0:
                            nc.vector.tensor_copy(
                                vp[:, s_, dh * 512:(dh + 1) * 512], ps)
                        else:
                            nc.vector.tensor_scalar_mul(
                                vp[:, s_, dh * 512:(dh + 1) * 512], ps,
                                1.0 / vgain)
                return vp

            if v_first:
                vp = do_v()
                kp = do_k()
            else:
                kp = do_k()
                vp = do_v()
            return kp, vp

        def attention(m, qp, kp, vp):
            """-> ctxT [128, 8(ci), 128(L)] bf16 (pre-out-proj context)."""
            f8 = kp.dtype == F8
            expt = exps.tile([P, KO, 512], BF16, tag="exp", name=f"expt_{m}")
            for s in range(KO):
                ps = psA.tile([P, 512], F32, tag="mm", name="ps_sc")
                for h in range(4):
                    if f8:
                        nc.tensor.matmul(
                            ps[:, h * P:(h + 1) * P],
                            kp[:, 2 * h:2 * h + 2, s * P:(s + 1) * P],
                            qp[:, 2 * h:2 * h + 2], start=True, stop=True,
                            perf_mode=DR)
                    else:
                        for dk in range(2):
                            nc.tensor.matmul(
                                ps[:, h * P:(h + 1) * P],
                                kp[:, 2 * h + dk, s * P:(s + 1) * P],
                                qp[:, 2 * h + dk], start=(dk == 0),
                                stop=(dk == 1))
                # fp8 kp and qp both carry x16 -> scores x256
                nc.scalar.activation(expt[:, s], ps, AF.Exp,
                                     scale=0.0625 / 256.0 if f8 else 0.0625)
            pss = pssum.tile([1, 512], F32, tag="cs", name="ps_sm")
            for s in range(KO):
                nc.tensor.matmul(pss, ones_cb, expt[:, s], start=(s == 0),
                                 stop=(s == KO - 1))
            inv = smalls.tile([1, 512], F32R, tag="inv512", name="inv_sm")
            with nc.allow_low_precision(reason="softmax reciprocal"):
                nc.vector.reciprocal(inv, pss)
            bc = bcast_row(inv, 512, BF16, tag="bcs", name=f"bcs_{m}")
            for s in range(KO):
                nc.vector.tensor_mul(expt[:, s], expt[:, s], bc)
            ctx = ctxs.tile([P, E], BF16, tag="ctx", name=f"ctx_{m}")
            for hh in range(2):
                ps = psA.tile([P, 512], F32, tag="mm", name="ps_av")
                for hi in range(2):
                    h = 2 * hh + hi
                    for s in range(KO):
                        nc.tensor.matmul(
                            ps[:, hi * 256:(hi + 1) * 256],
                            expt[:, s, h * P:(h + 1) * P],
                            vp[:, s, h * 256:(h + 1) * 256],
                            start=(s == 0), stop=(s == KO - 1))
                nc.scalar.activation(ctx[:, hh * 512:(hh + 1) * 512], ps,
                                     AF.Copy)
            if DEBUG and m == "tg":
                nc.sync.dma_start(dbg["d_ctx_tg"], ctx)
            ctxT = ctxs.tile([P, KO, L], BF16, tag="ctxT", name=f"ctxT_{m}")
            for ci in range(KO):
                pt = psT.tile([P, P], BF16, tag="tr", name="ps_tr")
                nc.tensor.transpose(pt, ctx[:, ci * P:(ci + 1) * P], ident)
                nc.vector.tensor_copy(ctxT[:, ci], pt)
            return ctxT

        def out_proj(m, ctxT, residual, master=False, out_pool=None,
                     out_dt=BF16, oscale=1.0, bias=None):
            wo = load_w(f"wo_{m}")
            return gemm_own(wo, ctxT, bias or bias_pp[f"bo_{m}"], f"o_{m}",
                            residual=residual, master=master,
                            out_pool=out_pool, out_dt=out_dt, oscale=oscale)

        def dump_feat(nm, t):
            if DEBUG:
                nc.sync.dma_start(
                    dbg[nm].rearrange("(ko p) r -> p ko r", p=P), t)

        def dump_plain(nm, t):
            if DEBUG:
                nc.sync.dma_start(dbg[nm], t)

        def pack_piece(inbuf, sb_tile):
            # NB: collective buffers must be bf16/f32 -- f32r payloads get
            # mantissa-squashed by the collective transport in this runtime.
            nc.sync.dma_start(
                inbuf.rearrange("(p a b) -> p a b", p=P, a=KO), sb_tile)

        def allgather(inbuf, outbuf):
            nc.gpsimd.collective_compute(
                "AllGather", mybir.AluOpType.bypass,
                replica_groups=GROUPS8,
                ins=[inbuf.opt()], outs=[outbuf.opt()])

        def unpack_gather(outbuf, name):
            # ci-major gathered layout [128, 8(ci), 8(rank), 128] so that
            # DoubleRow k-tile slices are contiguous in dim 1.
            t = gath.tile([P, KO, KO, L], outbuf.dtype, tag="gf", name=name)
            for r in range(KO):
                nc.sync.dma_start(
                    t[:, :, r],
                    outbuf[r].rearrange("(p a b) -> p a b", p=P, a=KO))
            return t

        # ---------- stage 0 ----------
        # critical input DMAs go first; the 21 small bias loads would
        # otherwise serialize ~12us of SP-queue time ahead of them.
        raw_to = load_raw("x_text_own", L, big=False)
        raw_g = load_raw("x_glob", E, big=True)
        w_tg0 = load_w("w_tg")
        load_biases()
        textn_own = normalize("x_text_own", L, acts, tag="nto", raw=raw_to)
        inv_g = norm_inv(raw_g, E)
        bc_g = bcast_row(inv_g, E, BF16, tag="bcn1024", name="bc_g")
        bcnT_g = transpose_inv(inv_g, "bcnT_g")

        # ---------- tg path ----------
        w_tg = w_tg0
        t_g_f8, t_g32 = gemm_own(w_tg, textn_own, bias_pp["b_tg"], "t_g",
                                 master=True, out_pool=pers2, out_dt=F8,
                                 oscale=32.0)
        wq_tg = load_w("wq_tg")
        qp_tg, _ = gemm_own(wq_tg, t_g_f8, bias_pp["bq_tg"], "qp_tg",
                            out_pool=qps, out_dt=F8, oscale=16.0,
                            in_scale=512.0, perf_mode=DR)
        kp_tg, vp_tg = kv_project_raw("tg", raw_g, bc_g, bcnT_g,
                                      bias_pp["kpos_tg"])
        ctxT_tg = attention("tg", qp_tg, kp_tg, vp_tg)
        gt_bf, _ = out_proj("tg", ctxT_tg, t_g32, master=True, out_dt=F8,
                            oscale=32.0, bias=bias_pp["bp_tg"])
        dump_feat("d_textn_own", textn_own)
        dump_feat("d_t_g", t_g_f8)
        dump_feat("d_qp_tg", qp_tg)
        dump_feat("d_kp_tg", kp_tg)
        dump_feat("d_vp_tg", vp_tg)
        dump_feat("d_gt", gt_bf)

        in1 = dram_p.tile([PIECE], F8, name="in1")
        out1 = dram_p.tile([KO, PIECE], F8, name="out1")
        pack_piece(in1, gt_bf)
        allgather(in1, out1)

        # ---------- tl path (overlaps gt gather) ----------
        raw_l = load_raw("x_loc", E, big=True)
        inv_l = norm_inv(raw_l, E)
        bc_l = bcast_row(inv_l, E, BF16, tag="bcn1024", name="bc_l")
        bcnT_l = transpose_inv(inv_l, "bcnT_l")
        w_tl = load_w("w_tl")
        t_l_f8, t_l32 = gemm_own(w_tl, textn_own, bias_pp["b_tl"], "t_l",
                                 master=True, out_pool=pers2, out_dt=F8,
                                 oscale=32.0)
        wq_tl = load_w("wq_tl")
        qp_tl, _ = gemm_own(wq_tl, t_l_f8, bias_pp["bq_tl"], "qp_tl",
                            out_pool=qps, out_dt=F8, oscale=16.0,
                            in_scale=512.0, perf_mode=DR)
        kp_tl, vp_tl = kv_project_raw("tl", raw_l, bc_l, bcnT_l,
                                      bias_pp["kpos_tl"])
        ctxT_tl = attention("tl", qp_tl, kp_tl, vp_tl)
        lt_f8, lt32 = out_proj("tl", ctxT_tl, t_l32, master=True,
                               out_pool=pers2, out_dt=F8, oscale=32.0)
        wq_ff = load_w("wq_ff")
        qp_ff, _ = gemm_own(wq_ff, lt_f8, bias_pp["bq_ff"], "qp_ff",
                            out_pool=qps, out_dt=F8, oscale=16.0,
                            in_scale=512.0, perf_mode=DR)
        # full-text norm scale (for t_r in the next window); the normalized
        # text itself is never materialized -- the scale commutes through
        # the t_r GEMM.
        raw_text = load_raw("x_text", E, big=True)
        inv_text = norm_inv(raw_text, E)
        bc_text = bcast_row(inv_text, E, F32R, tag="bct", name="bc_text")

        # ---------- ff MHA ----------
        gt_full = unpack_gather(out1, "gt_full")
        if DEBUG:
            for r in range(KO):
                nc.sync.dma_start(
                    dbg["d_gtf"][r].rearrange("(ko p) l -> p ko l", p=P),
                    gt_full[:, :, r])
        kp_ff, vp_ff = kv_project("ff", gt_full, 32.0)
        ctxT_ff = attention("ff", qp_ff, kp_ff, vp_ff)
        ff_bf, _ = out_proj("ff", ctxT_ff, lt32, bias=bias_pp["bp_ff"])
        dump_feat("d_lt", lt32)
        dump_feat("d_ff", ff_bf)

        in2 = dram_p.tile([PIECE], BF16, name="in2")
        out2 = dram_p.tile([KO, PIECE], BF16, name="out2")
        pack_piece(in2, ff_bf)
        allgather(in2, out2)

        # ---------- window 2 (overlaps ff gather): t_r + final prep ----------
        w_rep = load_w("w_rep")
        t_r = pers.tile([P, KO, E], BF16, name="t_r")
        for co in range(KO):
            for h4 in range(2):
                ps = psA.tile([P, 512], F32, tag="mm", name="ps_tr2")
                for ci in range(KO):
                    nc.tensor.matmul(ps, w_rep[:, ci, co * P:(co + 1) * P],
                                     raw_text[:, ci, h4 * 512:(h4 + 1) * 512],
                                     start=(ci == 0), stop=(ci == KO - 1))
                sl = t_r[:, co, h4 * 512:(h4 + 1) * 512]
                nc.vector.tensor_mul(sl, ps, bc_text[:, h4 * 512:(h4 + 1) * 512])
                nc.vector.tensor_scalar_add(sl, sl,
                                            bias_pp["b_rep"][:, co:co + 1])
        t_r_own, _ = gemm_own(w_rep, textn_own, bias_pp["b_rep"], "t_r_own",
                              out_dt=F8, oscale=32.0)
        wq_rt = load_w("wq_rt")
        qp_rt, _ = gemm_own(wq_rt, t_r_own, bias_pp["bq_rt"], "qp_rt",
                            out_pool=qps, out_dt=F8, oscale=16.0,
                            in_scale=512.0, perf_mode=DR)
        lfn = normalize("x_loc_grp", GRP, pers, tag="lfn", big=True)

        # ---------- rt MHA ----------
        ff_full = unpack_gather(out2, "ff_full")
        # fp8 x32 copy for the K/Q path (softmax washes fp8 noise); the
        # V path keeps the bf16 original.  Converted per rank chunk so it
        # pipelines with the unpack DMAs; V projection is emitted first so
        # the PE starts on bf16 V work while the conversion drains.
        ff_f8 = gath.tile([P, KO, KO, L], F8, tag="gf8", name="ff_f8")
        for r in range(KO):
            nc.gpsimd.tensor_scalar_mul(ff_f8[:, :, r], ff_full[:, :, r],
                                        32.0)
        kp_rt, vp_rt = kv_project("rt", ff_f8, 32.0, src_v=ff_full,
                                  v_scale=1.0, v_first=True)
        ctxT_rt = attention("rt", qp_rt, kp_rt, vp_rt)
        rt_bf, _ = out_proj("rt", ctxT_rt, None)
        dump_feat("d_t_r", t_r)
        dump_feat("d_rt", rt_bf)
        dump_feat("d_lfn", lfn)

        # ---------- final: full = rt @ t_r.T, cosine logits ----------
        # row-major full (for row norms): out[q(part), c] = sum_e rt[e,q] t_r[e,c]
        sq_scratch = finals.tile([P, 512], BF16, tag="fsq", name="fsq")
        frow = finals.tile([P, E], BF16, tag="frow", name="frow")
        acc = finals.tile([P, 2], F32, tag="acc2", name="acc_rn")
        for h4 in range(2):
            ps = psA.tile([P, 512], F32, tag="mm", name="ps_fr")
            for ci in range(KO):
                nc.tensor.matmul(ps, rt_bf[:, ci],
                                 t_r[:, ci, h4 * 512:(h4 + 1) * 512],
                                 start=(ci == 0), stop=(ci == KO - 1))
            nc.scalar.activation(frow[:, h4 * 512:(h4 + 1) * 512], ps, AF.Copy)
            nc.scalar.activation(sq_scratch, ps, AF.Square,
                                 accum_out=acc[:, h4:h4 + 1])
        rn = finals.tile([P, 1], F32, tag="rn", name="rn")
        nc.vector.tensor_add(rn, acc[:, 0:1], acc[:, 1:2])
        nc.scalar.sqrt(rn, rn)
        nc.vector.tensor_scalar_max(rn, rn, EPS)
        inv_q = finals.tile([P, 1], F32, tag="invq", name="inv_q")
        nc.vector.reciprocal(inv_q, rn)

        # feat-major fullT (logits lhsT) via PE transpose of full_row
        fullT = finals.tile([P, KO, L], BF16, tag="fullT", name="fullT")
        for cc in range(KO):
            pt = psT.tile([P, P], BF16, tag="tr", name="ps_ftr")
            nc.tensor.transpose(pt, frow[:, cc * P:(cc + 1) * P], ident)
            nc.vector.tensor_copy(fullT[:, cc], pt)

        dump_plain("d_frow", frow)
        lg = finals.tile([P, GRP], F32, tag="lg", name="lg")
        ps = psA.tile([P, 512], F32, tag="mm", name="ps_lg")
        for cc in range(KO):
            nc.tensor.matmul(ps[:, :GRP], fullT[:, cc], lfn[:, cc],
                             start=(cc == 0), stop=(cc == KO - 1))
        nc.vector.tensor_scalar_mul(lg, ps[:, :GRP], inv_q)
        nc.sync.dma_start(out_logits, lg)

    nc.compile()
    return nc


def make_in_maps(local_feat, global_feat, text_feat,
                 w_tl, b_tl, w_tg, b_tg, w_rep, b_rep,
                 pos_local, pos_global, mha_params):
    f32 = np.float32
    bf16 = ml_dtypes.bfloat16
    f8 = ml_dtypes.float8_e4m3
    textT = np.ascontiguousarray(text_feat.T.astype(bf16))
    locT = np.ascontiguousarray(local_feat.T.astype(bf16))
    shared = {
        "x_text": textT,
        "x_loc": np.ascontiguousarray(local_feat.T.astype(f8)),
        "x_glob": np.ascontiguousarray(global_feat.T.astype(f8)),
        "w_tl": np.ascontiguousarray(w_tl.T.astype(bf16)),
        "w_tg": np.ascontiguousarray(w_tg.T.astype(bf16)),
        "w_rep": np.ascontiguousarray(w_rep.T.astype(bf16)),
        "b_tl": b_tl.astype(f32), "b_tg": b_tg.astype(f32),
        "b_rep": b_rep.astype(f32),
    }
    wv_f, wo_f, bo_eff = {}, {}, {}
    for m, (wi, bi, wo, bo) in mha_params.items():
        # q/k/v weights in fp8 (x16 into the e4m3 sweet spot), except the
        # precision-critical rt block which stays bf16
        shared[f"wq_{m}"] = np.ascontiguousarray(
            (16.0 * wi[0 * E:1 * E].T).astype(f8))
        shared[f"wk_{m}"] = np.ascontiguousarray(
            (16.0 * wi[1 * E:2 * E].T).astype(f8))
        if m == "rt":
            shared[f"wv_{m}"] = np.ascontiguousarray(
                wi[2 * E:3 * E].T.astype(bf16))
        else:
            shared[f"wv_{m}"] = np.ascontiguousarray(
                (16.0 * wi[2 * E:3 * E].T).astype(f8))
        shared[f"wo_{m}"] = np.ascontiguousarray(wo.T.astype(bf16))
        shared[f"bq_{m}"] = bi[0 * E:1 * E].astype(f32)
        wv_f[m], wo_f[m] = wi[2 * E:3 * E], wo
        # V bias folded into output-projection bias: bo_eff = bo + wo @ bv
        bo_eff[m] = bo + wo @ bi[2 * E:3 * E]
    # The V projections run on pos-free / mean-shifted sources; each removed
    # constant c contributes wo @ (wv @ c) to the block's output bias:
    #   tl/tg: V source excludes pos_local/pos_global
    #   ff:    consumes gt' = gt - c_g   (c_g ~ mean over rows of gt)
    #   rt:    consumes ff' = ff - c_f
    bo_eff["tl"] = bo_eff["tl"] + wo_f["tl"] @ (wv_f["tl"] @ pos_local)
    bo_eff["tg"] = bo_eff["tg"] + wo_f["tg"] @ (wv_f["tg"] @ pos_global)
    c_g = bo_eff["tg"] + b_tg
    bo_eff["ff"] = bo_eff["ff"] + wo_f["ff"] @ (wv_f["ff"] @ c_g)
    c_f = bo_eff["ff"] + bo_eff["tl"] + b_tl
    bo_eff["rt"] = bo_eff["rt"] + wo_f["rt"] @ (wv_f["rt"] @ c_f)
    for m in mha_params:
        shared[f"bo_{m}"] = bo_eff[m].astype(f32)
    shared["bp_tg"] = (bo_eff["tg"] - c_g).astype(f32)
    shared["bp_ff"] = (bo_eff["ff"] - c_f).astype(f32)
    # host-folded K-projection pos terms (x16 to match fp8 kp scaling)
    shared["kpos_tl"] = (16.0 * (mha_params["tl"][0][E:2 * E] @
                                 pos_local)).astype(f32)
    shared["kpos_tg"] = (16.0 * (mha_params["tg"][0][E:2 * E] @
                                 pos_global)).astype(f32)

    in_maps = []
    for c in range(NCORES):
        g = c // 2
        m = dict(shared)
        m["x_text_own"] = np.ascontiguousarray(textT[:, c * L:(c + 1) * L])
        m["x_loc_grp"] = np.ascontiguousarray(locT[:, g * GRP:(g + 1) * GRP])
        in_maps.append(m)
    return in_maps


def kernel(local_feat, global_feat, text_feat,
           w_tl, b_tl, w_tg, b_tg, w_rep, b_rep,
           pos_local, pos_global,
           tl_wi, tl_bi, tl_wo, tl_bo,
           tg_wi, tg_bi, tg_wo, tg_bo,
           ff_wi, ff_bi, ff_wo, ff_bo,
           rt_wi, rt_bi, rt_wo, rt_bo,
           n_groups):
    assert int(n_groups) == 4
    if "nc" not in _CACHE:
        _CACHE["nc"] = build_nc()
    nc = _CACHE["nc"]
    mha_params = {
        "tl": (tl_wi, tl_bi, tl_wo, tl_bo),
        "tg": (tg_wi, tg_bi, tg_wo, tg_bo),
        "ff": (ff_wi, ff_bi, ff_wo, ff_bo),
        "rt": (rt_wi, rt_bi, rt_wo, rt_bo),
    }
    in_maps = make_in_maps(np.asarray(local_feat), np.asarray(global_feat),
                           np.asarray(text_feat),
                           np.asarray(w_tl), np.asarray(b_tl),
                           np.asarray(w_tg), np.asarray(b_tg),
                           np.asarray(w_rep), np.asarray(b_rep),
                           np.asarray(pos_local), np.asarray(pos_global),
                           {k: tuple(np.asarray(x) for x in v)
                            for k, v in mha_params.items()})
    res = run_bass_kernel_spmd(nc, in_maps, core_ids=list(range(NCORES)))
    _CACHE["last_results"] = res
    out = np.empty((4, GRP, GRP), dtype=np.float32)
    for c in range(NCORES):
        g, half = c // 2, c % 2
        out[g, half * L:(half + 1) * L, :] = res.results[c]["logits"]
    return out
